# revision 1
# baseline (speedup 1.0000x reference)
"""Bass/Trainium2 kernel for GQA decode attention (fused K-projection form).

Reference computation (per problem spec):
  x = x_pre[:, -1, :]                               # [16, 4096]
  xq = (x @ wq.T) -> [b, 32, 128]
  qt[b,h,:] = xq[b,h,:] @ wk[kv(h)*128:+128, :]     # [b, 32, 4096]
  scores = qt . x_pre / sqrt(128)                   # [b, 32, 2048]
  attn = softmax_t(scores)
  ctx[b,h,:] = sum_t attn[b,h,t] * x_pre[b,t,:]     # [b, 32, 4096]  (lazy-V)
  out[b,h,d] = sum_D ctx[b,h,D] * wv[kv(h)*128+d,D] # [b, 32, 128]
  y = out.flat @ wo.T                               # [16, 4096]

Sharding (8 cores): batch-parallel attention (2 batches/core) +
head-parallel projections (4 heads = 1 kv head/core), exchanged with two
AllToAll collectives. wo is column-sharded (contraction dim); host sums
the 8 partial y outputs.
"""

import math

import numpy as np

import concourse.bass as bass
import concourse.mybir as mybir
import concourse.tile as tile
from concourse import bacc
from concourse.bass_utils import run_bass_kernel_spmd
from concourse.masks import make_identity
from concourse.tile import add_dep_helper

F32 = mybir.dt.float32
NC = 8
BSZ = 16
SEQ = 2048
DIM = 4096
NH = 32
HD = 128
B_LOC = 2        # batches per core
HL = 4           # local heads per core
NT = SEQ // 128  # 16 t-tiles per batch
NDC = DIM // 128 # 32 D-chunks
SCALE = 1.0 / math.sqrt(HD)


def build_program(trace_label="", debug=False, nocc=False, skel=False):
    nc = bacc.Bacc("TRN2", target_bir_lowering=False, debug=False)

    xp = nc.dram_tensor("xp", [B_LOC, SEQ, DIM], F32, kind="ExternalInput")
    xl = nc.dram_tensor("xl", [BSZ, DIM], F32, kind="ExternalInput")
    wq = nc.dram_tensor("wq", [HL * HD, DIM], F32, kind="ExternalInput")
    wk = nc.dram_tensor("wk", [HD, DIM], F32, kind="ExternalInput")
    wv = nc.dram_tensor("wv", [HD, DIM], F32, kind="ExternalInput")
    wo = nc.dram_tensor("wo", [DIM, HL * HD], F32, kind="ExternalInput")
    y = nc.dram_tensor("y", [BSZ, DIM], F32, kind="ExternalOutput")
    if debug:
        dbg_q = nc.dram_tensor("dbg_q", [B_LOC * NH, DIM], F32,
                               kind="ExternalOutput")
        dbg_ctx = nc.dram_tensor("dbg_ctx", [BSZ * HL, DIM], F32,
                                 kind="ExternalOutput")
        dbg_sc = nc.dram_tensor("dbg_sc", [B_LOC * NH, 128], F32,
                                kind="ExternalOutput")
        dbg_xq = nc.dram_tensor("dbg_xq", [BSZ, HL * HD], F32,
                                kind="ExternalOutput")
        dbg_qs = nc.dram_tensor("dbg_qs", [BSZ, DIM], F32,
                                kind="ExternalOutput")

    rg = [list(range(NC))]

    with tile.TileContext(nc) as tc:
        with (
            tc.tile_pool(name="persist", bufs=1) as pers,
            tc.tile_pool(name="dram", bufs=1, space="DRAM") as dram,
        ):
            ident = pers.tile([128, 128], F32)
            make_identity(nc, ident)

            # DRAM exchange buffers
            a2a1_in = dram.tile([NC * B_LOC * HL, DIM], F32)   # [64, 4096]
            a2a1_out = dram.tile([NC * B_LOC * HL, DIM], F32)
            a2a2_in = dram.tile([NC * B_LOC * HL, DIM], F32)
            a2a2_out = dram.tile([NC * B_LOC * HL, DIM], F32)

            stage_dmas1 = []
            stage_dmas2 = []
            # ---------------- Phase 1: q-tilde for local heads, all batches
            with (
                tc.tile_pool(name="p1", bufs=2) as p1,
                tc.tile_pool(name="p1w", bufs=1) as p1w,
                tc.tile_pool(name="p1ps", bufs=2, space="PSUM") as p1ps,
            ):
                xl_sb = p1w.tile([BSZ, DIM], F32)
                nc.sync.dma_start(out=xl_sb, in_=xl[:, :])
                wk_sb = p1w.tile([HD, DIM], F32)
                nc.sync.dma_start(out=wk_sb, in_=wk[:, :])

                # xT: [128 D x 16 b] per D-chunk
                xT = p1w.tile([128, NDC * BSZ], F32)
                for c in range(NDC):
                    tp = p1ps.tile([128, BSZ], F32, tag="tp1")
                    nc.tensor.transpose(tp, xl_sb[:, c * 128:(c + 1) * 128],
                                        ident[0:BSZ, 0:BSZ])
                    nc.vector.tensor_copy(out=xT[:, c * BSZ:(c + 1) * BSZ], in_=tp)

                # wqT: per D-chunk c: [128 D x 512 hd]
                wqT = p1w.tile([128, NDC * HL * HD], F32)
                for m in range(HL):
                    wq_sb = p1.tile([128, DIM], F32, tag="wqnat")
                    nc.sync.dma_start(out=wq_sb, in_=wq[m * 128:(m + 1) * 128, :])
                    for c in range(NDC):
                        tp = p1ps.tile([128, 128], F32, tag="tp1")
                        nc.tensor.transpose(tp, wq_sb[:, c * 128:(c + 1) * 128],
                                            ident)
                        nc.vector.tensor_copy(
                            out=wqT[:, c * 512 + m * 128: c * 512 + (m + 1) * 128],
                            in_=tp)

                # xq = x @ wq_slice.T : accumulate over D-chunks -> [16 b, 512 hd]
                xq_ps = p1ps.tile([BSZ, HL * HD], F32, tag="xq")
                for c in range(NDC):
                    nc.tensor.matmul(xq_ps, xT[:, c * BSZ:(c + 1) * BSZ],
                                     wqT[:, c * 512:(c + 1) * 512],
                                     start=(c == 0), stop=(c == NDC - 1))
                xq_sb = p1w.tile([BSZ, HL * HD], F32)
                nc.vector.tensor_copy(out=xq_sb, in_=xq_ps)
                if debug:
                    nc.sync.dma_start(out=dbg_xq[:, :], in_=xq_sb)

                # xqT: [128 d x 16 b] per local head
                xqT = p1w.tile([128, HL * BSZ], F32)
                for m in range(HL):
                    tp = p1ps.tile([128, BSZ], F32, tag="tp1")
                    nc.tensor.transpose(tp, xq_sb[:, m * 128:(m + 1) * 128],
                                        ident[0:BSZ, 0:BSZ])
                    nc.vector.tensor_copy(out=xqT[:, m * BSZ:(m + 1) * BSZ], in_=tp)

                # qt[h] = xq[:,h,:] @ wk_kv  (scaled) -> staged [64, 4096]
                # row layout = h_loc*16 + b
                for m in range(HL):
                    qstage = p1.tile([BSZ, DIM], F32, tag="qstage")
                    for j in range(8):
                        q_ps = p1ps.tile([BSZ, 512], F32, tag="qps")
                        nc.tensor.matmul(q_ps, xqT[:, m * BSZ:(m + 1) * BSZ],
                                         wk_sb[:, j * 512:(j + 1) * 512],
                                         start=True, stop=True)
                        nc.scalar.mul(
                            out=qstage[:, j * 512:(j + 1) * 512],
                            in_=q_ps, mul=SCALE)
                    d = nc.sync.dma_start(
                        out=a2a1_in.rearrange("(r b h) d -> h r b d",
                                              r=NC, b=B_LOC)[m],
                        in_=qstage)
                    stage_dmas1.append(d)
                    if debug and m == 0:
                        nc.sync.dma_start(out=dbg_qs[:, :], in_=qstage)


            if not nocc:
                cc1 = nc.gpsimd.collective_compute(
                    "AllToAll", mybir.AluOpType.bypass,
                    ins=[a2a1_in.opt()], outs=[a2a1_out.opt()], replica_groups=rg)
                for d in stage_dmas1:
                    add_dep_helper(cc1.ins, d.ins, reason="a2a1 input ready")

            # qT per local batch: [128 D x 32 h] per D-chunk
            # a2a1_out row = src_r*8 + b_loc*4 + h_loc ; head = 4*src_r + h_loc
            qT = [pers.tile([128, NDC * NH], F32, tag=f"qT{b}", name=f"qT{b}")
                  for b in range(B_LOC)]
            with (
                tc.tile_pool(name="qnat", bufs=2) as qnatp,
                tc.tile_pool(name="qnps", bufs=2, space="PSUM") as qnps,
            ):
                for b in range(B_LOC):
                    qnat = qnatp.tile([NH, DIM], F32, tag="qnat")
                    d = nc.sync.dma_start(
                        out=qnat,
                        in_=a2a1_out.rearrange("(r b h) d -> b r h d",
                                               r=NC, b=B_LOC)[b])
                    if not nocc:
                        add_dep_helper(d.ins, cc1.ins, reason="a2a1 done")
                    if debug:
                        nc.sync.dma_start(out=dbg_q[b * NH:(b + 1) * NH, :],
                                          in_=qnat)
                    for c in range(NDC):
                        tp = qnps.tile([128, NH], F32, tag="tpq")
                        nc.tensor.transpose(tp, qnat[:, c * 128:(c + 1) * 128],
                                            ident[0:NH, 0:NH])
                        nc.vector.tensor_copy(
                            out=qT[b][:, c * NH:(c + 1) * NH], in_=tp)

            # ---------------- Phase 2: streaming attention per local batch
            with (
                tc.tile_pool(name="xpool", bufs=6) as xpool,
                tc.tile_pool(name="xtpool", bufs=1) as xtpool,
                tc.tile_pool(name="attn", bufs=3) as apool,
                tc.tile_pool(name="small", bufs=2) as smallp,
                tc.tile_pool(name="ctx_sb", bufs=1) as ctxsbp,
                tc.tile_pool(name="tps", bufs=3, space="PSUM") as tps,
                tc.tile_pool(name="scps", bufs=2, space="PSUM") as scps,
                tc.tile_pool(name="ctxps", bufs=1, space="PSUM") as ctxps,
            ):
                for b in range(B_LOC):
                    ctx_ps = ctxps.tile([128, 1024], F32, tag="ctx")
                    sums = smallp.tile([NH, 4], F32, tag="sums")
                    for ch in range(4):
                        xts = []
                        for tt in range(4):
                            t = ch * 4 + tt
                            x_sb = xpool.tile([128, DIM], F32, tag="x",
                                              name=f"x{b}_{t}")
                            nc.sync.dma_start(
                                out=x_sb, in_=xp[b, t * 128:(t + 1) * 128, :])
                            xts.append(x_sb)
                        xtt = xtpool.tile([128, NDC * 512], F32, tag="xt")
                        xv = xtt.rearrange("p (c t) -> p c t", c=NDC)
                        for tt in range(4):
                            for cg in range(NDC // 4):
                                tp = tps.tile([128, 512], F32, tag="tp2")
                                for cc in range(4):
                                    c = cg * 4 + cc
                                    nc.tensor.transpose(
                                        tp[:, cc * 128:(cc + 1) * 128],
                                        xts[tt][:, c * 128:(c + 1) * 128], ident)
                                nc.vector.tensor_copy(
                                    out=xv[:, cg * 4:(cg + 1) * 4,
                                           tt * 128:(tt + 1) * 128],
                                    in_=tp.rearrange("p (a q) -> p a q", a=4))
                        if skel:
                            continue
                        sc_ps = scps.tile([NH, 512], F32, tag="sc")
                        for c in range(NDC):
                            nc.tensor.matmul(sc_ps,
                                             qT[b][:, c * NH:(c + 1) * NH],
                                             xtt[:, c * 512:(c + 1) * 512],
                                             start=(c == 0), stop=(c == NDC - 1))
                        attn_sb = apool.tile([NH, 512], F32, tag="attn")
                        nc.scalar.activation(out=attn_sb, in_=sc_ps,
                                             func=mybir.ActivationFunctionType.Exp,
                                             accum_out=sums[:, ch:ch + 1])
                        at_ps = tps.tile([128, 512], F32, tag="tp2")
                        for tt in range(4):
                            nc.tensor.transpose(
                                at_ps[:, tt * NH:(tt + 1) * NH],
                                attn_sb[:, tt * 128:(tt + 1) * 128],
                                ident[0:NH, 0:NH])
                        at_sb = apool.tile([128, 4 * NH], F32, tag="attnT")
                        nc.vector.tensor_copy(out=at_sb, in_=at_ps[:, 0:4 * NH])
                        for tt in range(4):
                            for g in range(4):
                                for jj in (g, g + 4):
                                    nc.tensor.matmul(
                                        ctx_ps[g * 32:(g + 1) * 32,
                                               (jj // 4) * 512:(jj // 4 + 1) * 512],
                                        at_sb[:, tt * NH:(tt + 1) * NH],
                                        xts[tt][:, jj * 512:(jj + 1) * 512],
                                        start=(ch == 0 and tt == 0),
                                        stop=(ch == 3 and tt == 3),
                                        tile_position=(0, g * 32))
                    # finalize batch: 1/rowsum, scale, stage for exchange
                    ssum = smallp.tile([NH, 1], F32, tag="ssum")
                    nc.vector.reduce_sum(out=ssum, in_=sums,
                                         axis=mybir.AxisListType.X)
                    rsum = smallp.tile([128, 1], F32, tag="rsum")
                    nc.vector.reciprocal(out=rsum[0:NH], in_=ssum)
                    for g in range(1, 4):
                        nc.vector.tensor_copy(out=rsum[g * 32:(g + 1) * 32],
                                              in_=rsum[0:NH])
                    ctx_sb = ctxsbp.tile([NH, DIM], F32, tag="ctxsb")
                    for j in range(8):
                        g = j % 4
                        nc.vector.tensor_scalar_mul(
                            ctx_sb[:, j * 512:(j + 1) * 512],
                            ctx_ps[g * 32:(g + 1) * 32,
                                   (j // 4) * 512:(j // 4 + 1) * 512],
                            rsum[g * 32:(g + 1) * 32, 0:1])
                    # ship: dest rank r gets heads 4r..4r+3 ; row r*8 + b*4 + h
                    d = nc.sync.dma_start(
                        out=a2a2_in.rearrange("(r b h) d -> r b h d",
                                              r=NC, b=B_LOC)[:, b],
                        in_=ctx_sb)
                    stage_dmas2.append(d)

            if not nocc:
                cc2 = nc.gpsimd.collective_compute(
                    "AllToAll", mybir.AluOpType.bypass,
                    ins=[a2a2_in.opt()], outs=[a2a2_out.opt()], replica_groups=rg)
                for d in stage_dmas2:
                    add_dep_helper(cc2.ins, d.ins, reason="a2a2 input ready")

            # ---------------- Phase 3: output projection (local heads, all b)
            with (
                tc.tile_pool(name="p3", bufs=2) as p3,
                tc.tile_pool(name="p3w", bufs=1) as p3w,
                tc.tile_pool(name="p3ps", bufs=2, space="PSUM") as p3ps,
            ):
                # a2a2_out row = src_r*8 + b_loc*4 + h = b*4 + h  (b=2*src_r+b_loc)
                ctxg = p3w.tile([BSZ * HL, DIM], F32)
                d = nc.sync.dma_start(out=ctxg, in_=a2a2_out[:, :])
                if not nocc:
                    add_dep_helper(d.ins, cc2.ins, reason="a2a2 done")
                if debug:
                    nc.sync.dma_start(out=dbg_ctx[:, :], in_=ctxg)
                ctxgT = p3w.tile([128, NDC * BSZ * HL], F32)
                for c in range(NDC):
                    tp = p3ps.tile([128, 128], F32, tag="tp3")
                    nc.tensor.transpose(tp[:, 0:BSZ * HL],
                                        ctxg[:, c * 128:(c + 1) * 128],
                                        ident[0:BSZ * HL, 0:BSZ * HL])
                    nc.vector.tensor_copy(out=ctxgT[:, c * 64:(c + 1) * 64],
                                          in_=tp[:, 0:64])

                wv_sb = p3w.tile([HD, DIM], F32)
                nc.sync.dma_start(out=wv_sb, in_=wv[:, :])
                wvT = p3w.tile([128, DIM], F32)
                for c in range(NDC):
                    tp = p3ps.tile([128, 128], F32, tag="tp3")
                    nc.tensor.transpose(tp, wv_sb[:, c * 128:(c + 1) * 128], ident)
                    nc.vector.tensor_copy(out=wvT[:, c * 128:(c + 1) * 128], in_=tp)

                # out[b*4+h, d] accumulation over D-chunks
                op_ps = p3ps.tile([BSZ * HL, HD], F32, tag="op")
                for c in range(NDC):
                    nc.tensor.matmul(op_ps, ctxgT[:, c * 64:(c + 1) * 64],
                                     wvT[:, c * 128:(c + 1) * 128],
                                     start=(c == 0), stop=(c == NDC - 1))
                op_sb = p3w.tile([BSZ * HL, HD], F32)
                nc.vector.tensor_copy(out=op_sb, in_=op_ps)
                # outT [128 d x 64 (b*4+h)]
                otp = p3ps.tile([128, 128], F32, tag="tp3")
                nc.tensor.transpose(otp[:, 0:64], op_sb,
                                    ident[0:BSZ * HL, 0:BSZ * HL])
                outT = p3w.tile([128, BSZ * HL], F32)
                nc.vector.tensor_copy(out=outT, in_=otp[:, 0:64])

                # woT: per h_rel m: [128 d x 4096 j]
                woT = p3w.tile([128, HL * DIM], F32)
                for jt in range(NDC):
                    wo_sb = p3.tile([128, HL * HD], F32, tag="wonat")
                    nc.sync.dma_start(out=wo_sb, in_=wo[jt * 128:(jt + 1) * 128, :])
                    for m in range(HL):
                        tp = p3ps.tile([128, 128], F32, tag="tp3")
                        nc.tensor.transpose(tp, wo_sb[:, m * 128:(m + 1) * 128],
                                            ident)
                        nc.vector.tensor_copy(
                            out=woT[:, m * DIM + jt * 128: m * DIM + (jt + 1) * 128],
                            in_=tp)

                # y partial [16 b, 4096 j]
                y_sb = p3w.tile([BSZ, DIM], F32)
                oT = outT.rearrange("p (b h) -> p h b", h=HL)
                for jc in range(8):
                    y_ps = p3ps.tile([BSZ, 512], F32, tag="yps")
                    for m in range(HL):
                        nc.tensor.matmul(y_ps, oT[:, m, :],
                                         woT[:, m * DIM + jc * 512:
                                             m * DIM + (jc + 1) * 512],
                                         start=(m == 0), stop=(m == HL - 1))
                    nc.vector.tensor_copy(out=y_sb[:, jc * 512:(jc + 1) * 512],
                                          in_=y_ps)
                nc.sync.dma_start(out=y[:, :], in_=y_sb)

    nc.finalize()
    return nc


_PROGRAM_CACHE = {}


def kernel(x_pre, wq, wk, wv, wo, _trace=False, _tmpdir=None):
    x_pre = np.ascontiguousarray(np.asarray(x_pre, dtype=np.float32))
    wq = np.asarray(wq, dtype=np.float32)
    wk = np.asarray(wk, dtype=np.float32)
    wv = np.asarray(wv, dtype=np.float32)
    wo = np.asarray(wo, dtype=np.float32)

    if "nc" not in _PROGRAM_CACHE:
        _PROGRAM_CACHE["nc"] = build_program()
    nc = _PROGRAM_CACHE["nc"]

    xl = np.ascontiguousarray(x_pre[:, -1, :])
    in_maps = []
    for i in range(NC):
        in_maps.append({
            "xp": np.ascontiguousarray(x_pre[2 * i:2 * i + 2]),
            "xl": xl,
            "wq": np.ascontiguousarray(wq[512 * i:512 * (i + 1), :]),
            "wk": np.ascontiguousarray(wk[128 * i:128 * (i + 1), :]),
            "wv": np.ascontiguousarray(wv[128 * i:128 * (i + 1), :]),
            "wo": np.ascontiguousarray(wo[:, 512 * i:512 * (i + 1)]),
        })

    kwargs = {}
    if _trace:
        kwargs = dict(trace=True, trace_cores=[0])
    if _tmpdir is not None:
        kwargs["tmpdir"] = _tmpdir
    res = run_bass_kernel_spmd(nc, in_maps, core_ids=list(range(NC)), **kwargs)
    y = np.zeros((BSZ, DIM), np.float32)
    for i in range(NC):
        y += res.results[i]["y"]
    if _trace:
        print("HW exec time:", res.exec_time_ns, "ns")
    return y.reshape(BSZ, 1, DIM)



# revision 18
# speedup vs baseline: 3.3889x; 3.3889x over previous
"""Bass/Trainium2 kernel for GQA decode attention (fused K-projection form).

Reference computation:
  x = x_pre[:, -1, :]                               # [16, 4096]
  xq = (x @ wq.T) -> [b, 32, 128]
  qt[b,h,:] = xq[b,h,:] @ wk[kv(h)*128:+128, :]     # [b, 32, 4096]
  scores = qt . x_pre / sqrt(128)                   # [b, 32, 2048]
  attn = softmax_t(scores)
  ctx[b,h,:] = sum_t attn[b,h,t] * x_pre[b,t,:]     # [b, 32, 4096]  (lazy-V)
  out[b,h,d] = sum_D ctx[b,h,D] * wv[kv(h)*128+d,D] # [b, 32, 128]
  y = out.flat @ wo.T                               # [16, 4096]

Sharding (8 cores): batch-parallel attention (2 batches/core) +
head-parallel projections (4 heads = 1 kv group/core), exchanged with
AllToAll collectives.  All device data is bf16 (f32 PSUM accumulation);
weights are pre-transposed on the host into the layouts the PE consumes,
and the big matmuls are arranged stationary-heavy (large lhsT, narrow
moving operand) so PE streaming cost is minimized.
"""

import math

import numpy as np
import ml_dtypes

import concourse.bass as bass
import concourse.mybir as mybir
import concourse.tile as tile
from concourse import bacc
from concourse.bass_utils import run_bass_kernel_spmd
from concourse.masks import make_identity
from concourse.tile import add_dep_helper

F32 = mybir.dt.float32
BF16 = mybir.dt.bfloat16
NPBF = ml_dtypes.bfloat16

NC = 8
BSZ = 16
SEQ = 2048
DIM = 4096
NH = 32
HD = 128
B_LOC = 2        # batches per core
HL = 4           # local heads per core (= one kv group)
NT = SEQ // 128  # 16 t-tiles per batch
NDC = DIM // 128 # 32 D-chunks
SCALE = 1.0 / math.sqrt(HD)


def build_program(debug=False, nocc=False, noattn=False, notrans=False, nocopy=False):
    nc = bacc.Bacc("TRN2", target_bir_lowering=False, debug=False)

    xp = nc.dram_tensor("xp", [B_LOC, SEQ, DIM], BF16, kind="ExternalInput")
    # xlT[p, c, b] = x_pre[b, -1, c*128+p]
    xlT = nc.dram_tensor("xlT", [128, NDC * BSZ], BF16, kind="ExternalInput")
    # wqT[p, c*512 + h*128 + o] = wq[512r + h*128 + o, c*128 + p]
    wqT = nc.dram_tensor("wqT", [128, NDC * HL * HD], BF16,
                         kind="ExternalInput")
    # wk_s = wk[128r:128(r+1), :] * SCALE   (natural [d, D])
    wk = nc.dram_tensor("wk", [HD, DIM], BF16, kind="ExternalInput")
    # wvT[p, c*128 + d] = wv[128r + d, c*128 + p]
    wvT = nc.dram_tensor("wvT", [128, NDC * HD], BF16, kind="ExternalInput")
    # woT[p, h*4096 + jc*128 + j] = wo[jc*128 + j, 512r + h*128 + p]
    woT = nc.dram_tensor("woT", [128, HL * DIM], BF16, kind="ExternalInput")
    # yT[p, jc*16 + b*8 + s] = y_partial[2s+b, jc*128+p]
    yT = nc.dram_tensor("yT", [128, NDC * B_LOC * NC], F32,
                        kind="ExternalOutput")
    if debug:
        dbg_xq = nc.dram_tensor("dbg_xq", [128, 64], BF16, kind="ExternalOutput")
        dbg_qt = nc.dram_tensor("dbg_qt", [64, DIM], BF16, kind="ExternalOutput")
        dbg_qtT = nc.dram_tensor("dbg_qtT", [128, B_LOC * NDC * NH],
                                 BF16, kind="ExternalOutput")
        dbg_ctx = nc.dram_tensor("dbg_ctx", [NH, B_LOC * DIM], BF16,
                                 kind="ExternalOutput")
        dbg_out = nc.dram_tensor("dbg_out", [NH, B_LOC * HD], BF16,
                                 kind="ExternalOutput")

    rg = [list(range(NC))]
    vs_engines = None  # round-robin copy engines, set below

    with tile.TileContext(nc) as tc:
        with (
            tc.tile_pool(name="persist", bufs=1) as pers,
            tc.tile_pool(name="dram", bufs=1, space="DRAM") as dram,
        ):
            ident = pers.tile([128, 128], BF16)
            make_identity(nc, ident)
            ones_bf = pers.tile([128, 1], BF16)
            nc.vector.memset(ones_bf, 1.0)

            a2a1_in = dram.tile([NC * HL * B_LOC, DIM], BF16)
            a2a1_out = dram.tile([NC * HL * B_LOC, DIM], BF16)
            a2a2_in = [dram.tile([NC * HL, DIM], BF16, name=f"a2a2i{b}")
                       for b in range(B_LOC)]
            a2a2_out = [dram.tile([NC * HL, DIM], BF16, name=f"a2a2o{b}")
                        for b in range(B_LOC)]

            # ---------------- Phase A: projections (head-sharded, all b)
            qtT = [pers.tile([128, NDC * NH], BF16, name=f"qtT{b}")
                   for b in range(B_LOC)]
            stage1 = []
            with (
                tc.tile_pool(name="pA", bufs=1) as pA,
                tc.tile_pool(name="pAq", bufs=2) as pAq,
                tc.tile_pool(name="pAps", bufs=2, space="PSUM") as pAps,
            ):
                xlT_sb = pA.tile([128, NDC * BSZ], BF16)
                nc.sync.dma_start(out=xlT_sb, in_=xlT[:, :])
                wqT_sb = pA.tile([128, NDC * HL * HD], BF16)
                nc.sync.dma_start(out=wqT_sb, in_=wqT[:, :])
                wk_sb = pA.tile([HD, DIM], BF16)
                nc.sync.dma_start(out=wk_sb, in_=wk[:, :])

                # xqT[p=o, h*16+b] accumulated over D-chunks
                # NOTE: accumulation chains sharing a PSUM bank must be
                # contiguous in program order: a start=True matmul clears
                # has_written for the whole bank, so interleaved chains
                # lose their first contribution.  h-outer keeps each chain
                # contiguous.
                xqT_ps = pAps.tile([128, HL * BSZ], F32, tag="xqps")
                for h in range(HL):
                    for c in range(NDC):
                        nc.tensor.matmul(
                            xqT_ps[:, h * BSZ:(h + 1) * BSZ],
                            wqT_sb[:, c * 512 + h * 128: c * 512 + (h + 1) * 128],
                            xlT_sb[:, c * BSZ:(c + 1) * BSZ],
                            start=(c == 0), stop=(c == NDC - 1))
                xqT_sb = pA.tile([128, HL * BSZ], BF16)
                nc.vector.tensor_copy(out=xqT_sb, in_=xqT_ps)
                if debug:
                    nc.sync.dma_start(out=dbg_xq[:, :], in_=xqT_sb)

                # qt[h*16+b, D] = sum_d xqT[d, (h,b)] * wk_s[d, D]
                qstage = pA.tile([HL * BSZ, DIM], BF16)
                for j in range(8):
                    qt_ps = pAps.tile([HL * BSZ, 512], F32, tag="qtps")
                    nc.tensor.matmul(qt_ps, xqT_sb,
                                     wk_sb[:, j * 512:(j + 1) * 512],
                                     start=True, stop=True)
                    if j % 2 == 0:
                        nc.vector.tensor_copy(
                            out=qstage[:, j * 512:(j + 1) * 512], in_=qt_ps)
                    else:
                        nc.scalar.copy(
                            out=qstage[:, j * 512:(j + 1) * 512], in_=qt_ps)
                if debug:
                    nc.sync.dma_start(out=dbg_qt[:, :], in_=qstage)

                # stage rows h*16 + r*2 + b  ->  a2a1_in row r*8 + h*2 + b
                qv = qstage.rearrange("(h r b) d -> h b r d", h=HL, r=NC)
                av = a2a1_in.rearrange("(r h b) d -> h b r d", r=NC, h=HL)
                for h in range(HL):
                    for bl in range(B_LOC):
                        d = nc.sync.dma_start(out=av[h, bl], in_=qv[h, bl])
                        stage1.append(d)

            cc1 = None
            if not nocc:
                cc1 = nc.gpsimd.collective_compute(
                    "AllToAll", mybir.AluOpType.bypass,
                    ins=[a2a1_in.opt()], outs=[a2a1_out.opt()],
                    replica_groups=rg)
                for d in stage1:
                    add_dep_helper(cc1.ins, d.ins, reason="a2a1 input ready")

            # qtT[b]: [128 D, c*32 + h] from a2a1_out rows s*8 + h*2 + b
            with (
                tc.tile_pool(name="qn", bufs=2) as qn,
                tc.tile_pool(name="qnps", bufs=2, space="PSUM") as qnps,
            ):
                anv = a2a1_out.rearrange("(s h b) d -> b s h d", s=NC, h=HL)
                for b in range(B_LOC):
                    qnat = qn.tile([NH, DIM], BF16, tag="qnat")
                    d = nc.sync.dma_start(out=qnat, in_=anv[b])
                    if cc1 is not None:
                        add_dep_helper(d.ins, cc1.ins, reason="a2a1 done")
                    for cg in range(2):
                        tp = qnps.tile([128, 512], BF16, tag="qtp")
                        for k in range(16):
                            c = cg * 16 + k
                            nc.tensor.transpose(
                                tp[:, k * 32:(k + 1) * 32],
                                qnat[:, c * 128:(c + 1) * 128],
                                ident[0:NH, 0:NH])
                        nc.vector.tensor_copy(
                            out=qtT[b][:, cg * 512:(cg + 1) * 512], in_=tp)
                    if debug:
                        nc.sync.dma_start(
                            out=dbg_qtT[:, b * NDC * NH:(b + 1) * NDC * NH],
                            in_=qtT[b])

            # ---------------- Phase B: streaming attention per local batch
            cc2 = [None, None]
            with (
                tc.tile_pool(name="xpool", bufs=4) as xpool,
                tc.tile_pool(name="xTpool", bufs=3) as xTpool,
                tc.tile_pool(name="attn", bufs=3) as apool,
                tc.tile_pool(name="small", bufs=2) as smallp,
                tc.tile_pool(name="ctxsb", bufs=1) as ctxsbp,
                tc.tile_pool(name="tps", bufs=2, space="PSUM") as tps,
                tc.tile_pool(name="scps", bufs=2, space="PSUM") as scps,
                tc.tile_pool(name="ctxps", bufs=1, space="PSUM") as ctxps,
                tc.tile_pool(name="sumps", bufs=1, space="PSUM") as sumps,
                tc.tile_pool(name="miscps", bufs=1, space="PSUM") as miscps,
                # phase C pools (open alongside B so slot 0 overlaps b1)
                tc.tile_pool(name="pC", bufs=2) as pC,
                tc.tile_pool(name="pCw", bufs=1) as pCw,
            ):
                fps = miscps
                pCps = miscps
                yps = miscps
                def xt_copy(g, out, in_):
                    # GPSIMD cannot read PSUM; split PSUM->SBUF copies
                    # between DVE and ACT.
                    if g in (1, 3, 5):
                        nc.scalar.copy(out=out, in_=in_)
                    else:
                        nc.vector.tensor_copy(out=out, in_=in_)
                for b in range(B_LOC):
                    ctx_ps = ctxps.tile([128, NDC * NH], F32, tag="ctx")
                    sumT_ps = sumps.tile([NH, 1], F32, tag="sumT")
                    for tt in range(NT):
                        x_sb = xpool.tile([128, DIM], BF16, tag="x",
                                          name=f"x{b}_{tt}")
                        nc.sync.dma_start(
                            out=x_sb, in_=xp[b, tt * 128:(tt + 1) * 128, :])
                        xT_sb = xTpool.tile([128, DIM], BF16, tag="xT")
                        for g in range(0 if notrans else 8):
                            tp = tps.tile([128, 512], BF16, tag="xtp")
                            for k in range(4):
                                c = g * 4 + k
                                nc.tensor.transpose(
                                    tp[:, k * 128:(k + 1) * 128],
                                    x_sb[:, c * 128:(c + 1) * 128], ident)
                            xt_copy(g, xT_sb[:, g * 512:(g + 1) * 512], tp)
                        if noattn:
                            continue
                        # scoresT[t, h] accumulated over D-chunks.
                        # Padded to a full bank so the two rotation bufs land
                        # in different banks (start=True wipes a whole bank).
                        sc_full = scps.tile([128, 512], F32, tag="sc")
                        sc_ps = sc_full[:, 0:NH]
                        for c in range(NDC):
                            nc.tensor.matmul(
                                sc_ps,
                                xT_sb[:, c * 128:(c + 1) * 128],
                                qtT[b][:, c * NH:(c + 1) * NH],
                                start=(c == 0), stop=(c == NDC - 1))
                        at_sb = apool.tile([128, NH], BF16, tag="at")
                        nc.scalar.activation(
                            out=at_sb, in_=sc_ps,
                            func=mybir.ActivationFunctionType.Exp)
                        # rowsum: sumT[h] += sum_t at[t, h]
                        nc.tensor.matmul(sumT_ps, at_sb, ones_bf,
                                         start=(tt == 0), stop=(tt == NT - 1))
                        # ctxT[D, h] += x[t, D]^T @ at[t, h]
                        # 32 chunk-chains share 2 PSUM banks; start=True only
                        # on the first chunk of each bank (wipes the bank),
                        # the rest of tt==0 overwrites via has_written=0.
                        bank_start = [None, None]
                        for c in range(NDC):
                            mm = nc.tensor.matmul(
                                ctx_ps[:, c * NH:(c + 1) * NH],
                                x_sb[:, c * 128:(c + 1) * 128],
                                at_sb,
                                start=(tt == 0 and c % 16 == 0),
                                stop=(tt == NT - 1),
                                skip_group_check=True)
                            if tt == 0:
                                if c % 16 == 0:
                                    bank_start[c // 16] = mm
                                else:
                                    add_dep_helper(
                                        mm.ins, bank_start[c // 16].ins,
                                        reason="bank wipe first")
                    # finalize batch b
                    sumT_sb = smallp.tile([NH, 1], F32, tag="ssum")
                    nc.vector.tensor_copy(out=sumT_sb, in_=sumT_ps)
                    recip = smallp.tile([NH, 1], F32, tag="recip")
                    nc.vector.reciprocal(out=recip, in_=sumT_sb)
                    ctxT_sb = ctxsbp.tile([128, NDC * NH], BF16, tag="ctxT")
                    for half in range(2):
                        nc.vector.tensor_copy(
                            out=ctxT_sb[:, half * 512:(half + 1) * 512],
                            in_=ctx_ps[:, half * 512:(half + 1) * 512])
                    ctx_sb = ctxsbp.tile([NH, DIM], BF16, tag="ctxn")
                    for g in range(8):
                        tp2f = fps.tile([128, 512], BF16, tag="ctp")
                        tp2 = tp2f[0:NH]
                        for k in range(4):
                            c = g * 4 + k
                            nc.tensor.transpose(
                                tp2[:, k * 128:(k + 1) * 128],
                                ctxT_sb[:, c * NH:(c + 1) * NH],
                                ident)
                        nc.vector.tensor_scalar_mul(
                            ctx_sb[:, g * 512:(g + 1) * 512], tp2, recip)
                    if debug:
                        nc.sync.dma_start(
                            out=dbg_ctx[:, b * DIM:(b + 1) * DIM], in_=ctx_sb)
                    d = nc.sync.dma_start(out=a2a2_in[b][:, :], in_=ctx_sb)
                    if not nocc:
                        cc2[b] = nc.gpsimd.collective_compute(
                            "AllToAll", mybir.AluOpType.bypass,
                            ins=[a2a2_in[b].opt()], outs=[a2a2_out[b].opt()],
                            replica_groups=rg)
                        add_dep_helper(cc2[b].ins, d.ins,
                                       reason="a2a2 input ready")

                # ---------------- Phase C: output projection per batch slot
                wvT_sb = pCw.tile([128, NDC * HD], BF16)
                nc.sync.dma_start(out=wvT_sb, in_=wvT[:, :])
                woT_sb = pCw.tile([128, HL * DIM], BF16)
                nc.sync.dma_start(out=woT_sb, in_=woT[:, :])
                yT_sb = pCw.tile([128, NDC * B_LOC * NC], F32)
                for b in range(B_LOC):
                    ctxg = pC.tile([NH, DIM], BF16, tag="ctxg")
                    d = nc.sync.dma_start(out=ctxg, in_=a2a2_out[b][:, :])
                    if cc2[b] is not None:
                        add_dep_helper(d.ins, cc2[b].ins, reason="a2a2 done")
                    ctxgT = pC.tile([128, NDC * NH], BF16, tag="ctxgT")
                    for cg in range(2):
                        tp = pCps.tile([128, 512], BF16, tag="ctp")
                        for k in range(16):
                            c = cg * 16 + k
                            nc.tensor.transpose(
                                tp[:, k * 32:(k + 1) * 32],
                                ctxg[:, c * 128:(c + 1) * 128],
                                ident[0:NH, 0:NH])
                        nc.vector.tensor_copy(
                            out=ctxgT[:, cg * 512:(cg + 1) * 512], in_=tp)
                    # out[(s,h), d] = sum_D ctxgT[D, (s,h)] * wvT[D, d]
                    op_ps = pCps.tile([NH, HD], F32, tag="ctp")
                    for c in range(NDC):
                        nc.tensor.matmul(op_ps,
                                         ctxgT[:, c * NH:(c + 1) * NH],
                                         wvT_sb[:, c * 128:(c + 1) * 128],
                                         start=(c == 0), stop=(c == NDC - 1))
                    op_sb = pC.tile([NH, HD], BF16, tag="opsb")
                    nc.vector.tensor_copy(out=op_sb, in_=op_ps)
                    if debug:
                        nc.sync.dma_start(
                            out=dbg_out[:, b * HD:(b + 1) * HD], in_=op_sb)
                    otp = pCps.tile([128, NH], BF16, tag="ctp")
                    nc.tensor.transpose(otp, op_sb, ident[0:NH, 0:NH])
                    outT = pC.tile([128, NH], BF16, tag="outT")
                    nc.vector.tensor_copy(out=outT, in_=otp)
                    # yT[j, s] = sum_h sum_d woT[d, (h, jc, j)] * outT[d, (s, h)]
                    ov = outT.rearrange("p (s h) -> p h s", h=HL)
                    y_ps = yps.tile([128, NDC * NC], F32, tag="ctp")
                    for jc in range(NDC):
                        for h in range(HL):
                            nc.tensor.matmul(
                                y_ps[:, jc * NC:(jc + 1) * NC],
                                woT_sb[:, h * DIM + jc * 128:
                                       h * DIM + (jc + 1) * 128],
                                ov[:, h, :],
                                start=(h == 0), stop=(h == HL - 1))
                    yv = yT_sb.rearrange("p (jc b s) -> b p jc s", jc=NDC,
                                         b=B_LOC)
                    nc.vector.tensor_copy(
                        out=yv[b],
                        in_=y_ps.rearrange("p (jc s) -> p jc s", jc=NDC))
                nc.sync.dma_start(out=yT[:, :], in_=yT_sb)

    nc.finalize()
    return nc


_PROGRAM_CACHE = {}


def _prep_inputs(x_pre, wq, wk, wv, wo):
    """Shard + cast + pre-transpose on host. Returns in_maps for 8 cores."""
    xlT_full = np.ascontiguousarray(
        x_pre[:, -1, :].T.astype(NPBF))                    # [4096, 16]
    xlT_full = xlT_full.reshape(NDC, 128, BSZ).transpose(1, 0, 2)  # [128,c,b]
    xlT_flat = np.ascontiguousarray(xlT_full.reshape(128, NDC * BSZ))

    wk_s = (wk * SCALE).astype(NPBF)
    in_maps = []
    for r in range(NC):
        # wqT[p, c, h, o] = wq[512r + h*128 + o, c*128 + p]
        wq_sl = wq[512 * r:512 * (r + 1), :].astype(NPBF)   # [512, 4096] (h,o)xD
        wqT_r = wq_sl.reshape(HL, 128, NDC, 128).transpose(3, 2, 0, 1)
        wqT_r = np.ascontiguousarray(wqT_r.reshape(128, NDC * HL * HD))
        # wvT[p, c, d] = wv[128r + d, c*128 + p]
        wv_sl = wv[128 * r:128 * (r + 1), :].astype(NPBF)   # [128 d, 4096 D]
        wvT_r = wv_sl.reshape(128, NDC, 128).transpose(2, 1, 0)
        wvT_r = np.ascontiguousarray(wvT_r.reshape(128, NDC * HD))
        # woT[p, h, jc, j] = wo[jc*128 + j, 512r + h*128 + p]
        wo_sl = wo[:, 512 * r:512 * (r + 1)].astype(NPBF)   # [4096 j, 512 o]
        woT_r = wo_sl.reshape(NDC, 128, HL, 128).transpose(3, 2, 0, 1)
        woT_r = np.ascontiguousarray(woT_r.reshape(128, HL * DIM))
        in_maps.append({
            "xp": np.ascontiguousarray(x_pre[2 * r:2 * r + 2].astype(NPBF)),
            "xlT": xlT_flat,
            "wqT": wqT_r,
            "wk": np.ascontiguousarray(wk_s[128 * r:128 * (r + 1), :]),
            "wvT": wvT_r,
            "woT": woT_r,
        })
    return in_maps


def kernel(x_pre, wq, wk, wv, wo, _trace=False, _tmpdir=None, _debug=False):
    x_pre = np.asarray(x_pre, dtype=np.float32)
    wq = np.asarray(wq, dtype=np.float32)
    wk = np.asarray(wk, dtype=np.float32)
    wv = np.asarray(wv, dtype=np.float32)
    wo = np.asarray(wo, dtype=np.float32)

    key = "nc_dbg" if _debug else "nc"
    if key not in _PROGRAM_CACHE:
        _PROGRAM_CACHE[key] = build_program(debug=_debug)
        _PROGRAM_CACHE["nc"] = _PROGRAM_CACHE[key]
    nc = _PROGRAM_CACHE[key]

    in_maps = _prep_inputs(x_pre, wq, wk, wv, wo)

    kwargs = {}
    if _trace:
        kwargs = dict(trace=True, trace_cores=[0])
    if _tmpdir is not None:
        kwargs["tmpdir"] = _tmpdir
    res = run_bass_kernel_spmd(nc, in_maps, core_ids=list(range(NC)), **kwargs)

    y = np.zeros((BSZ, DIM), np.float64)
    for r in range(NC):
        yT_r = np.asarray(res.results[r]["yT"], np.float32)
        yT_r = yT_r.reshape(128, NDC, B_LOC, NC)
        # y[2s+b, jc*128+p] += yT_r[p, jc, b, s]
        y += yT_r.transpose(3, 2, 1, 0).reshape(BSZ, DIM)
    if _debug:
        _PROGRAM_CACHE["dbg"] = res
    if _trace:
        print("HW exec time:", res.exec_time_ns, "ns")
    return y.astype(np.float32).reshape(BSZ, 1, DIM)


# revision 25
# speedup vs baseline: 4.5211x; 1.3341x over previous
"""Bass/Trainium2 kernel for GQA decode attention (fused K-projection form).

Reference computation:
  x = x_pre[:, -1, :]                               # [16, 4096]
  xq = (x @ wq.T) -> [b, 32, 128]
  qt[b,h,:] = xq[b,h,:] @ wk[kv(h)*128:+128, :]     # [b, 32, 4096]
  scores = qt . x_pre / sqrt(128)                   # [b, 32, 2048]
  attn = softmax_t(scores)
  ctx[b,h,:] = sum_t attn[b,h,t] * x_pre[b,t,:]     # [b, 32, 4096]  (lazy-V)
  out[b,h,d] = sum_D ctx[b,h,D] * wv[kv(h)*128+d,D] # [b, 32, 128]
  y = out.flat @ wo.T                               # [16, 4096]

Sharding (8 cores): batch-parallel attention (2 batches/core) +
head-parallel projections (4 heads = 1 kv group/core), exchanged with
AllToAll collectives.  All device data is bf16 (f32 PSUM accumulation);
weights are pre-transposed on the host into the layouts the PE consumes,
and the big matmuls are arranged stationary-heavy (large lhsT, narrow
moving operand) so PE streaming cost is minimized.
"""

import math

import numpy as np
import ml_dtypes

import concourse.bass as bass
import concourse.mybir as mybir
import concourse.tile as tile
from concourse import bacc
from concourse.bass_utils import run_bass_kernel_spmd
from concourse.masks import make_identity
from concourse.tile import add_dep_helper

F32 = mybir.dt.float32
BF16 = mybir.dt.bfloat16
NPBF = ml_dtypes.bfloat16

NC = 8
BSZ = 16
SEQ = 2048
DIM = 4096
NH = 32
HD = 128
B_LOC = 2        # batches per core
HL = 4           # local heads per core (= one kv group)
NT = SEQ // 128  # 16 t-tiles per batch
NDC = DIM // 128 # 32 D-chunks
SCALE = 1.0 / math.sqrt(HD)


def build_program(debug=False, nocc=False, noattn=False, notrans=False, nocopy=False):
    nc = bacc.Bacc("TRN2", target_bir_lowering=False, debug=False)

    xp = nc.dram_tensor("xp", [B_LOC, SEQ, DIM], BF16, kind="ExternalInput")
    # xlT[p, c, b] = x_pre[b, -1, c*128+p]
    xlT = nc.dram_tensor("xlT", [128, NDC * BSZ], BF16, kind="ExternalInput")
    # wqT[p, c*512 + h*128 + o] = wq[512r + h*128 + o, c*128 + p]
    wqT = nc.dram_tensor("wqT", [128, NDC * HL * HD], BF16,
                         kind="ExternalInput")
    # wk_s = wk[128r:128(r+1), :] * SCALE   (natural [d, D])
    wk = nc.dram_tensor("wk", [HD, DIM], BF16, kind="ExternalInput")
    # wvT[p, c*128 + d] = wv[128r + d, c*128 + p]
    wvT = nc.dram_tensor("wvT", [128, NDC * HD], BF16, kind="ExternalInput")
    # woT[p, h*4096 + jc*128 + j] = wo[jc*128 + j, 512r + h*128 + p]
    woT = nc.dram_tensor("woT", [128, HL * DIM], BF16, kind="ExternalInput")
    # yT[p, jc*16 + b*8 + s] = y_partial[2s+b, jc*128+p]
    yT = nc.dram_tensor("yT", [128, NDC * B_LOC * NC], F32,
                        kind="ExternalOutput")
    if debug:
        dbg_xq = nc.dram_tensor("dbg_xq", [128, 64], BF16, kind="ExternalOutput")
        dbg_qt = nc.dram_tensor("dbg_qt", [64, DIM], BF16, kind="ExternalOutput")
        dbg_qtT = nc.dram_tensor("dbg_qtT", [128, B_LOC * NDC * NH],
                                 BF16, kind="ExternalOutput")
        dbg_ctx = nc.dram_tensor("dbg_ctx", [NH, B_LOC * DIM], BF16,
                                 kind="ExternalOutput")
        dbg_out = nc.dram_tensor("dbg_out", [NH, B_LOC * HD], BF16,
                                 kind="ExternalOutput")

    rg = [list(range(NC))]
    vs_engines = None  # round-robin copy engines, set below

    with tile.TileContext(nc) as tc:
        with (
            tc.tile_pool(name="persist", bufs=1) as pers,
            tc.tile_pool(name="dram", bufs=1, space="DRAM") as dram,
            tc.tile_pool(name="xpool", bufs=8) as xpool,
            tc.tile_pool(name="xTpool", bufs=5) as xTpool,
            tc.tile_pool(name="attn", bufs=3) as apool,
            tc.tile_pool(name="small", bufs=2) as smallp,
            tc.tile_pool(name="ctxsb", bufs=1) as ctxsbp,
            tc.tile_pool(name="pC", bufs=1) as pC,
            tc.tile_pool(name="pCw", bufs=1) as pCw,
            tc.tile_pool(name="tps", bufs=3, space="PSUM") as tps,
            tc.tile_pool(name="scps", bufs=1, space="PSUM") as scps,
            tc.tile_pool(name="ctxps", bufs=1, space="PSUM") as ctxps,
            tc.tile_pool(name="sumps", bufs=1, space="PSUM") as sumps,
            tc.tile_pool(name="miscps", bufs=1, space="PSUM") as miscps,
        ):
            fps = miscps
            pCps = miscps
            yps = miscps
            ident = pers.tile([128, 128], BF16)
            make_identity(nc, ident)
            ones_bf = pers.tile([128, 1], BF16)
            nc.vector.memset(ones_bf, 1.0)

            a2a1_in = dram.tile([NC * HL * B_LOC, DIM], BF16)
            a2a1_out = dram.tile([NC * HL * B_LOC, DIM], BF16)
            a2a2_in = [dram.tile([NC * HL, DIM], BF16, name=f"a2a2i{b}")
                       for b in range(B_LOC)]
            a2a2_out = [dram.tile([NC * HL, DIM], BF16, name=f"a2a2o{b}")
                        for b in range(B_LOC)]

            # ---------------- Phase A: projections (head-sharded, all b)
            qtT = [pers.tile([128, NDC * NH], BF16, name=f"qtT{b}")
                   for b in range(B_LOC)]
            stage1 = []
            with (
                tc.tile_pool(name="pA", bufs=1) as pA,
                tc.tile_pool(name="pAw", bufs=2) as pAw,
            ):
                xlT_sb = pA.tile([128, NDC * BSZ], BF16)
                nc.sync.dma_start(out=xlT_sb, in_=xlT[:, :])
                wq_pieces = []
                for q in range(4):
                    wq_q = pAw.tile([128, 8 * HL * HD], BF16, tag="wqq",
                                    name=f"wqq{q}")
                    nc.sync.dma_start(
                        out=wq_q, in_=wqT[:, q * 4096:(q + 1) * 4096])
                    wq_pieces.append(wq_q)
                wk_sb = pA.tile([HD, DIM], BF16)
                nc.sync.dma_start(out=wk_sb, in_=wk[:, :])

                # xqT[p=o, h*16+b] accumulated over D-chunks
                # NOTE: accumulation chains sharing a PSUM bank must be
                # contiguous in program order: a start=True matmul clears
                # has_written for the whole bank, so interleaved chains
                # lose their first contribution.  h-outer keeps each chain
                # contiguous.
                xqT_psf = scps.tile([128, 512], F32, tag="sc")
                xqT_ps = xqT_psf[:, 0:HL * BSZ]
                first_mm = None
                for q in range(4):
                    for h in range(HL):
                        for k in range(8):
                            c = q * 8 + k
                            mm = nc.tensor.matmul(
                                xqT_ps[:, h * BSZ:(h + 1) * BSZ],
                                wq_pieces[q][:, k * 512 + h * 128:
                                             k * 512 + (h + 1) * 128],
                                xlT_sb[:, c * BSZ:(c + 1) * BSZ],
                                start=(q == 0 and h == 0 and k == 0),
                                stop=(q == 3 and k == 7),
                                skip_group_check=True)
                            if first_mm is None:
                                first_mm = mm
                            elif q == 0:
                                add_dep_helper(mm.ins, first_mm.ins,
                                               reason="bank wipe first")
                xqT_sb = pA.tile([128, HL * BSZ], BF16)
                nc.vector.tensor_copy(out=xqT_sb, in_=xqT_ps)
                if debug:
                    nc.sync.dma_start(out=dbg_xq[:, :], in_=xqT_sb)

                # qt[h*16+b, D] = sum_d xqT[d, (h,b)] * wk_s[d, D]
                qstage = pA.tile([HL * BSZ, DIM], BF16)
                for j in range(8):
                    if j % 2 == 0:
                        qt_ps = scps.tile([HL * BSZ, 512], F32, tag="sc")
                    else:
                        qt_ps = miscps.tile([HL * BSZ, 512], F32, tag="ctp")
                    nc.tensor.matmul(qt_ps, xqT_sb,
                                     wk_sb[:, j * 512:(j + 1) * 512],
                                     start=True, stop=True)
                    if j % 2 == 0:
                        nc.vector.tensor_copy(
                            out=qstage[:, j * 512:(j + 1) * 512], in_=qt_ps)
                    else:
                        nc.scalar.copy(
                            out=qstage[:, j * 512:(j + 1) * 512], in_=qt_ps)
                if debug:
                    nc.sync.dma_start(out=dbg_qt[:, :], in_=qstage)

                # stage rows h*16 + r*2 + b  ->  a2a1_in row r*8 + h*2 + b
                qv = qstage.rearrange("(h r b) d -> h b r d", h=HL, r=NC)
                av = a2a1_in.rearrange("(r h b) d -> h b r d", r=NC, h=HL)
                for h in range(HL):
                    for bl in range(B_LOC):
                        d = nc.scalar.dma_start(out=av[h, bl], in_=qv[h, bl])
                        stage1.append(d)

            cc1 = None
            if not nocc:
                cc1 = nc.gpsimd.collective_compute(
                    "AllToAll", mybir.AluOpType.bypass,
                    ins=[a2a1_in.opt()], outs=[a2a1_out.opt()],
                    replica_groups=rg)
                for d in stage1:
                    add_dep_helper(cc1.ins, d.ins, reason="a2a1 input ready")

            # qtT[b]: [128 D, c*32 + h] from a2a1_out rows s*8 + h*2 + b
            with tc.tile_pool(name="qn", bufs=2) as qn:
                anv = a2a1_out.rearrange("(s h b) d -> b s h d", s=NC, h=HL)
                for b in range(B_LOC):
                    qnat = qn.tile([NH, DIM], BF16, tag="qnat")
                    d = nc.scalar.dma_start(out=qnat, in_=anv[b])
                    if cc1 is not None:
                        add_dep_helper(d.ins, cc1.ins, reason="a2a1 done")
                    for cg in range(2):
                        tp = scps.tile([128, 512], BF16, tag="sc")
                        for k in range(16):
                            c = cg * 16 + k
                            nc.tensor.transpose(
                                tp[:, k * 32:(k + 1) * 32],
                                qnat[:, c * 128:(c + 1) * 128],
                                ident[0:NH, 0:NH])
                        nc.vector.tensor_copy(
                            out=qtT[b][:, cg * 512:(cg + 1) * 512], in_=tp)
                    if debug:
                        nc.sync.dma_start(
                            out=dbg_qtT[:, b * NDC * NH:(b + 1) * NDC * NH],
                            in_=qtT[b])

            # ---------------- Phase B: streaming attention per local batch
            cc2 = [None, None]
            if True:
                xdmas = []
                def xt_copy(g, out, in_):
                    # GPSIMD cannot read PSUM; split PSUM->SBUF copies
                    # between DVE and ACT.
                    if g == 1:
                        nc.scalar.copy(out=out, in_=in_)
                    else:
                        nc.vector.tensor_copy(out=out, in_=in_)
                for b in range(B_LOC):
                    ctx_ps = ctxps.tile([128, NDC * NH], F32, tag="ctx")
                    sumT_ps = sumps.tile([NH, 1], F32, tag="sumT")
                    for tt in range(NT):
                        x_sb = xpool.tile([128, DIM], BF16, tag="x",
                                          name=f"x{b}_{tt}")
                        xd = nc.sync.dma_start(
                            out=x_sb, in_=xp[b, tt * 128:(tt + 1) * 128, :])
                        if len(xdmas) == 2 and stage1:
                            # let the tiny a2a1 staging transfers through the
                            # shared DMA engines before bulk prefetch
                            add_dep_helper(xd.ins, stage1[-1].ins,
                                           reason="stage before prefetch")
                        xdmas.append(xd)
                        xT_sb = xTpool.tile([128, DIM], BF16, tag="xT")
                        for g in range(0 if notrans else 4):
                            tp = tps.tile([128, 1024], BF16, tag="xtp")
                            for k in range(8):
                                c = g * 8 + k
                                nc.tensor.transpose(
                                    tp[:, k * 128:(k + 1) * 128],
                                    x_sb[:, c * 128:(c + 1) * 128], ident)
                            xt_copy(g, xT_sb[:, g * 1024:(g + 1) * 1024], tp)
                        if noattn:
                            continue
                        # scoresT[t, h] accumulated over D-chunks.
                        # Padded to a full bank so the two rotation bufs land
                        # in different banks (start=True wipes a whole bank).
                        sc_full = scps.tile([128, 512], F32, tag="sc")
                        sc_ps = sc_full[:, 0:NH]
                        for c in range(NDC):
                            nc.tensor.matmul(
                                sc_ps,
                                xT_sb[:, c * 128:(c + 1) * 128],
                                qtT[b][:, c * NH:(c + 1) * NH],
                                start=(c == 0), stop=(c == NDC - 1))
                        at_sb = apool.tile([128, NH], BF16, tag="at")
                        nc.scalar.activation(
                            out=at_sb, in_=sc_ps,
                            func=mybir.ActivationFunctionType.Exp)
                        # rowsum: sumT[h] += sum_t at[t, h]
                        nc.tensor.matmul(sumT_ps, at_sb, ones_bf,
                                         start=(tt == 0), stop=(tt == NT - 1))
                        # ctxT[D, h] += x[t, D]^T @ at[t, h]
                        # 32 chunk-chains share 2 PSUM banks; start=True only
                        # on the first chunk of each bank (wipes the bank),
                        # the rest of tt==0 overwrites via has_written=0.
                        bank_start = [None, None]
                        for c in range(NDC):
                            mm = nc.tensor.matmul(
                                ctx_ps[:, c * NH:(c + 1) * NH],
                                x_sb[:, c * 128:(c + 1) * 128],
                                at_sb,
                                start=(tt == 0 and c % 16 == 0),
                                stop=(tt == NT - 1),
                                skip_group_check=True)
                            if tt == 0:
                                if c % 16 == 0:
                                    bank_start[c // 16] = mm
                                else:
                                    add_dep_helper(
                                        mm.ins, bank_start[c // 16].ins,
                                        reason="bank wipe first")
                    # finalize batch b
                    sumT_sb = smallp.tile([NH, 1], F32, tag="ssum")
                    nc.vector.tensor_copy(out=sumT_sb, in_=sumT_ps)
                    recip = smallp.tile([NH, 1], F32, tag="recip")
                    nc.vector.reciprocal(out=recip, in_=sumT_sb)
                    ctxT_sb = ctxsbp.tile([128, NDC * NH], BF16, tag="ctxT")
                    for half in range(2):
                        nc.vector.tensor_copy(
                            out=ctxT_sb[:, half * 512:(half + 1) * 512],
                            in_=ctx_ps[:, half * 512:(half + 1) * 512])
                    ctx_sb = ctxsbp.tile([NH, DIM], BF16, tag="ctxn")
                    for g in range(4):
                        tp2f = tps.tile([128, 1024], BF16, tag="xtp")
                        tp2 = tp2f[0:NH]
                        for k in range(8):
                            c = g * 8 + k
                            nc.tensor.transpose(
                                tp2[:, k * 128:(k + 1) * 128],
                                ctxT_sb[:, c * NH:(c + 1) * NH],
                                ident)
                        nc.vector.tensor_scalar_mul(
                            ctx_sb[:, g * 1024:(g + 1) * 1024], tp2, recip)
                    if debug:
                        nc.sync.dma_start(
                            out=dbg_ctx[:, b * DIM:(b + 1) * DIM], in_=ctx_sb)
                    d = nc.gpsimd.dma_start(out=a2a2_in[b][:, :], in_=ctx_sb)
                    if not nocc:
                        cc2[b] = nc.gpsimd.collective_compute(
                            "AllToAll", mybir.AluOpType.bypass,
                            ins=[a2a2_in[b].opt()], outs=[a2a2_out[b].opt()],
                            replica_groups=rg)
                        add_dep_helper(cc2[b].ins, d.ins,
                                       reason="a2a2 input ready")

                # ---------------- Phase C: output projection per batch slot
                wvT_sb = pCw.tile([128, NDC * HD], BF16)
                dwv = nc.sync.dma_start(out=wvT_sb, in_=wvT[:, :])
                woT_sb = pCw.tile([128, HL * DIM], BF16)
                dwo = nc.sync.dma_start(out=woT_sb, in_=woT[:, :])
                # keep the big phase-C weight loads out of the x-stream's way
                add_dep_helper(dwv.ins, xdmas[NT + 4].ins, reason="late wv")
                add_dep_helper(dwo.ins, xdmas[NT + 8].ins, reason="late wo")
                yT_sb = pCw.tile([128, NDC * B_LOC * NC], F32)
                for b in range(B_LOC):
                    ctxg = pC.tile([NH, DIM], BF16, tag="ctxg")
                    d = nc.gpsimd.dma_start(out=ctxg, in_=a2a2_out[b][:, :])
                    if cc2[b] is not None:
                        add_dep_helper(d.ins, cc2[b].ins, reason="a2a2 done")
                    ctxgT = pC.tile([128, NDC * NH], BF16, tag="ctxgT")
                    tpg = tps.tile([128, 1024], BF16, tag="xtp")
                    for c in range(NDC):
                        nc.tensor.transpose(
                            tpg[:, c * 32:(c + 1) * 32],
                            ctxg[:, c * 128:(c + 1) * 128],
                            ident[0:NH, 0:NH])
                    nc.vector.tensor_copy(out=ctxgT, in_=tpg)
                    # out[(s,h), d] = sum_D ctxgT[D, (s,h)] * wvT[D, d]
                    op_ps = pCps.tile([NH, HD], F32, tag="ctp")
                    for c in range(NDC):
                        nc.tensor.matmul(op_ps,
                                         ctxgT[:, c * NH:(c + 1) * NH],
                                         wvT_sb[:, c * 128:(c + 1) * 128],
                                         start=(c == 0), stop=(c == NDC - 1))
                    op_sb = pC.tile([NH, HD], BF16, tag="opsb")
                    nc.vector.tensor_copy(out=op_sb, in_=op_ps)
                    if debug:
                        nc.sync.dma_start(
                            out=dbg_out[:, b * HD:(b + 1) * HD], in_=op_sb)
                    otp = pCps.tile([128, NH], BF16, tag="ctp")
                    nc.tensor.transpose(otp, op_sb, ident[0:NH, 0:NH])
                    outT = pC.tile([128, NH], BF16, tag="outT")
                    nc.vector.tensor_copy(out=outT, in_=otp)
                    # yT[j, s] = sum_h sum_d woT[d, (h, jc, j)] * outT[d, (s, h)]
                    ov = outT.rearrange("p (s h) -> p h s", h=HL)
                    y_ps = yps.tile([128, NDC * NC], F32, tag="ctp")
                    for jc in range(NDC):
                        for h in range(HL):
                            nc.tensor.matmul(
                                y_ps[:, jc * NC:(jc + 1) * NC],
                                woT_sb[:, h * DIM + jc * 128:
                                       h * DIM + (jc + 1) * 128],
                                ov[:, h, :],
                                start=(h == 0), stop=(h == HL - 1))
                    yv = yT_sb.rearrange("p (jc b s) -> b p jc s", jc=NDC,
                                         b=B_LOC)
                    nc.vector.tensor_copy(
                        out=yv[b],
                        in_=y_ps.rearrange("p (jc s) -> p jc s", jc=NDC))
                nc.sync.dma_start(out=yT[:, :], in_=yT_sb)

    nc.finalize()
    return nc


_PROGRAM_CACHE = {}


def _prep_inputs(x_pre, wq, wk, wv, wo):
    """Shard + cast + pre-transpose on host. Returns in_maps for 8 cores."""
    xlT_full = np.ascontiguousarray(
        x_pre[:, -1, :].T.astype(NPBF))                    # [4096, 16]
    xlT_full = xlT_full.reshape(NDC, 128, BSZ).transpose(1, 0, 2)  # [128,c,b]
    xlT_flat = np.ascontiguousarray(xlT_full.reshape(128, NDC * BSZ))

    wk_s = (wk * SCALE).astype(NPBF)
    in_maps = []
    for r in range(NC):
        # wqT[p, c, h, o] = wq[512r + h*128 + o, c*128 + p]
        wq_sl = wq[512 * r:512 * (r + 1), :].astype(NPBF)   # [512, 4096] (h,o)xD
        wqT_r = wq_sl.reshape(HL, 128, NDC, 128).transpose(3, 2, 0, 1)
        wqT_r = np.ascontiguousarray(wqT_r.reshape(128, NDC * HL * HD))
        # wvT[p, c, d] = wv[128r + d, c*128 + p]
        wv_sl = wv[128 * r:128 * (r + 1), :].astype(NPBF)   # [128 d, 4096 D]
        wvT_r = wv_sl.reshape(128, NDC, 128).transpose(2, 1, 0)
        wvT_r = np.ascontiguousarray(wvT_r.reshape(128, NDC * HD))
        # woT[p, h, jc, j] = wo[jc*128 + j, 512r + h*128 + p]
        wo_sl = wo[:, 512 * r:512 * (r + 1)].astype(NPBF)   # [4096 j, 512 o]
        woT_r = wo_sl.reshape(NDC, 128, HL, 128).transpose(3, 2, 0, 1)
        woT_r = np.ascontiguousarray(woT_r.reshape(128, HL * DIM))
        in_maps.append({
            "xp": np.ascontiguousarray(x_pre[2 * r:2 * r + 2].astype(NPBF)),
            "xlT": xlT_flat,
            "wqT": wqT_r,
            "wk": np.ascontiguousarray(wk_s[128 * r:128 * (r + 1), :]),
            "wvT": wvT_r,
            "woT": woT_r,
        })
    return in_maps


def kernel(x_pre, wq, wk, wv, wo, _trace=False, _tmpdir=None, _debug=False):
    x_pre = np.asarray(x_pre, dtype=np.float32)
    wq = np.asarray(wq, dtype=np.float32)
    wk = np.asarray(wk, dtype=np.float32)
    wv = np.asarray(wv, dtype=np.float32)
    wo = np.asarray(wo, dtype=np.float32)

    key = "nc_dbg" if _debug else "nc"
    if key not in _PROGRAM_CACHE:
        _PROGRAM_CACHE[key] = build_program(debug=_debug)
        _PROGRAM_CACHE["nc"] = _PROGRAM_CACHE[key]
    nc = _PROGRAM_CACHE[key]

    in_maps = _prep_inputs(x_pre, wq, wk, wv, wo)

    kwargs = {}
    if _trace:
        kwargs = dict(trace=True, trace_cores=[0])
    if _tmpdir is not None:
        kwargs["tmpdir"] = _tmpdir
    res = run_bass_kernel_spmd(nc, in_maps, core_ids=list(range(NC)), **kwargs)

    y = np.zeros((BSZ, DIM), np.float64)
    for r in range(NC):
        yT_r = np.asarray(res.results[r]["yT"], np.float32)
        yT_r = yT_r.reshape(128, NDC, B_LOC, NC)
        # y[2s+b, jc*128+p] += yT_r[p, jc, b, s]
        y += yT_r.transpose(3, 2, 1, 0).reshape(BSZ, DIM)
    if _debug:
        _PROGRAM_CACHE["dbg"] = res
    if _trace:
        print("HW exec time:", res.exec_time_ns, "ns")
    return y.astype(np.float32).reshape(BSZ, 1, DIM)


# revision 47
# speedup vs baseline: 4.7255x; 1.0452x over previous
"""Bass/Trainium2 kernel for GQA decode attention (fused K-projection form).

Reference computation:
  x = x_pre[:, -1, :]                               # [16, 4096]
  xq = (x @ wq.T) -> [b, 32, 128]
  qt[b,h,:] = xq[b,h,:] @ wk[kv(h)*128:+128, :]     # [b, 32, 4096]
  scores = qt . x_pre / sqrt(128)                   # [b, 32, 2048]
  attn = softmax_t(scores)
  ctx[b,h,:] = sum_t attn[b,h,t] * x_pre[b,t,:]     # [b, 32, 4096]  (lazy-V)
  out[b,h,d] = sum_D ctx[b,h,D] * wv[kv(h)*128+d,D] # [b, 32, 128]
  y = out.flat @ wo.T                               # [16, 4096]

Sharding (8 cores): batch-parallel attention (2 batches/core) +
head-parallel projections (4 heads = 1 kv group/core), exchanged with
AllToAll collectives.  All device data is bf16 (f32 PSUM accumulation);
weights are pre-transposed on the host into the layouts the PE consumes,
and the big matmuls are arranged stationary-heavy (large lhsT, narrow
moving operand) so PE streaming cost is minimized.
"""

import math

import numpy as np
import ml_dtypes

import concourse.bass as bass
import concourse.mybir as mybir
import concourse.tile as tile
from concourse import bacc
from concourse.bass_utils import run_bass_kernel_spmd
from concourse.masks import make_identity
from concourse.tile import add_dep_helper

F32 = mybir.dt.float32
BF16 = mybir.dt.bfloat16
NPBF = ml_dtypes.bfloat16

NC = 8
BSZ = 16
SEQ = 2048
DIM = 4096
NH = 32
HD = 128
B_LOC = 2        # batches per core
HL = 4           # local heads per core (= one kv group)
N_KV = 8
NT = SEQ // 128  # 16 t-tiles per batch
NDC = DIM // 128 # 32 D-chunks
SCALE = 1.0 / math.sqrt(HD)


def build_program(debug=False, nocc=False, noattn=False, notrans=False, nocopy=False):
    nc = bacc.Bacc("TRN2", target_bir_lowering=False, debug=False)

    xp = nc.dram_tensor("xp", [B_LOC, SEQ, DIM], BF16, kind="ExternalInput")
    # xlT[p, c, b] = x_pre[b, -1, c*128+p]
    xlT = nc.dram_tensor("xlT", [128, NDC * BSZ], BF16, kind="ExternalInput")
    # wqT[p, c*512 + h*128 + o] = wq[512r + h*128 + o, c*128 + p]
    wqT = nc.dram_tensor("wqT", [128, NDC * HL * HD], BF16,
                         kind="ExternalInput")
    # wk_s = wk * SCALE  (full, natural [kv*128+d, D])
    wk = nc.dram_tensor("wk", [N_KV * HD, DIM], BF16, kind="ExternalInput")
    # wvT[p, c*128 + d] = wv[128r + d, c*128 + p]
    wvT = nc.dram_tensor("wvT", [128, NDC * HD], BF16, kind="ExternalInput")
    # woT[p, h*4096 + jc*128 + j] = wo[jc*128 + j, 512r + h*128 + p]
    woT = nc.dram_tensor("woT", [128, HL * DIM], BF16, kind="ExternalInput")
    # yT[p, jc*16 + b*8 + s] = y_partial[2s+b, jc*128+p]
    yT = nc.dram_tensor("yT", [128, NDC * B_LOC * NC], F32,
                        kind="ExternalOutput")
    if debug:
        dbg_xq = nc.dram_tensor("dbg_xq", [128, 64], BF16, kind="ExternalOutput")
        dbg_qt = nc.dram_tensor("dbg_qt", [64, DIM], BF16, kind="ExternalOutput")
        dbg_qtT = nc.dram_tensor("dbg_qtT", [128, B_LOC * NDC * NH],
                                 BF16, kind="ExternalOutput")
        dbg_ctx = nc.dram_tensor("dbg_ctx", [NH, B_LOC * DIM], BF16,
                                 kind="ExternalOutput")
        dbg_out = nc.dram_tensor("dbg_out", [NH, B_LOC * HD], BF16,
                                 kind="ExternalOutput")

    rg = [list(range(NC))]
    vs_engines = None  # round-robin copy engines, set below

    with tile.TileContext(nc) as tc:
        with (
            tc.tile_pool(name="persist", bufs=1) as pers,
            tc.tile_pool(name="dram", bufs=1, space="DRAM") as dram,
            tc.tile_pool(name="xpool", bufs=6) as xpool,
            tc.tile_pool(name="xTpool", bufs=3) as xTpool,
            tc.tile_pool(name="attn", bufs=3) as apool,
            tc.tile_pool(name="small", bufs=2) as smallp,
            tc.tile_pool(name="ctxsb", bufs=1) as ctxsbp,
            tc.tile_pool(name="pC", bufs=1) as pC,
            tc.tile_pool(name="pCw", bufs=1) as pCw,
            tc.tile_pool(name="tps", bufs=3, space="PSUM") as tps,
            tc.tile_pool(name="scps", bufs=1, space="PSUM") as scps,
                        tc.tile_pool(name="ctxps", bufs=1, space="PSUM") as ctxps,
            tc.tile_pool(name="sumps", bufs=1, space="PSUM") as sumps,
            tc.tile_pool(name="miscps", bufs=1, space="PSUM") as miscps,
        ):
            fps = miscps
            pCps = miscps
            yps = miscps
            ident = pers.tile([128, 128], BF16)
            make_identity(nc, ident)
            ones_bf = pers.tile([128, 1], BF16)
            nc.vector.memset(ones_bf, 1.0)

            a2a1_in = dram.tile([BSZ, 512], BF16)
            a2a1_out = dram.tile([BSZ, 512], BF16)
            a2a2_in = [dram.tile([NC * HL, DIM], BF16, name=f"a2a2i{b}")
                       for b in range(B_LOC)]
            a2a2_out = [dram.tile([NC * HL, DIM], BF16, name=f"a2a2o{b}")
                        for b in range(B_LOC)]

            # ---------------- Phase A: xq (head-sharded) -> tiny AllToAll
            qtT_all = pers.tile([128, B_LOC * NDC * NH], BF16, name="qtTall")
            qtT = [qtT_all[:, b * NDC * NH:(b + 1) * NDC * NH]
                   for b in range(B_LOC)]
            stage1 = []
            with (
                tc.tile_pool(name="pA", bufs=1) as pA,
                tc.tile_pool(name="pAw", bufs=2) as pAw,
            ):
                xlT_sb = pA.tile([128, NDC * BSZ], BF16)
                nc.sync.dma_start(out=xlT_sb, in_=xlT[:, :])
                wq_pieces = []
                for q in range(4):
                    wq_q = pAw.tile([128, 8 * HL * HD], BF16, tag="wqq",
                                    name=f"wqq{q}")
                    nc.sync.dma_start(
                        out=wq_q, in_=wqT[:, q * 4096:(q + 1) * 4096])
                    wq_pieces.append(wq_q)
                # xq[b, o] for the local 512-wide o-slice; one PSUM chain
                xq_psf = scps.tile([128, 512], F32, tag="sc")
                xq_ps = xq_psf[0:BSZ]
                for q in range(4):
                    for k in range(8):
                        c = q * 8 + k
                        nc.tensor.matmul(
                            xq_ps,
                            xlT_sb[:, c * BSZ:(c + 1) * BSZ],
                            wq_pieces[q][:, k * 512:(k + 1) * 512],
                            start=(c == 0), stop=(c == NDC - 1))
                xq_sb = pA.tile([BSZ, 512], BF16)
                nc.scalar.copy(out=xq_sb, in_=xq_ps)
                d = nc.scalar.dma_start(out=a2a1_in[:, :], in_=xq_sb)
                stage1.append(d)

            cc1 = None
            if not nocc:
                cc1 = nc.gpsimd.collective_compute(
                    "AllToAll", mybir.AluOpType.bypass,
                    ins=[a2a1_in.opt()], outs=[a2a1_out.opt()],
                    replica_groups=rg)
                for d in stage1:
                    add_dep_helper(cc1.ins, d.ins, reason="a2a1 input ready")

            # qtT[b][p=D, c*32+h] = sum_d xq[2r+b, h*128+d] * wk_s[h*128+d, c*128+p]
            with (
                tc.tile_pool(name="qn", bufs=1) as qn,
                tc.tile_pool(name="wkp", bufs=8) as wkp,
            ):
                # wk pieces per kv group, streamed (SP queue, after wqT)
                wk_dmas = []
                wk_pieces = []
                for kv in range(N_KV):
                    wkq = wkp.tile([HD, DIM], BF16, tag="wkp",
                                   name=f"wk{kv}")
                    dk = nc.scalar.dma_start(
                        out=wkq, in_=wk[kv * HD:(kv + 1) * HD, :])

                    wk_dmas.append(dk)
                    wk_pieces.append(wkq)
                xq_loc = qn.tile([B_LOC, DIM], BF16)
                xql_dmas = []
                av = a2a1_out.rearrange("(sq bl) o -> bl sq o", sq=NC)
                for bl in range(B_LOC):
                    d = nc.scalar.dma_start(
                        out=xq_loc[bl:bl + 1].rearrange(
                            "p (sq o) -> p sq o", sq=NC),
                        in_=av[bl])
                    xql_dmas.append(d)
                    if cc1 is not None:
                        add_dep_helper(d.ins, cc1.ins, reason="a2a1 done")
                # xqT2[p=d, 2*hg+bl] via PE transposes
                xqT2_psf = miscps.tile([128, 512], BF16, tag="ctp")
                for c in range(NDC):
                    nc.tensor.transpose(
                        xqT2_psf[:, c * 2:(c + 1) * 2],
                        xq_loc[:, c * 128:(c + 1) * 128],
                        ident[0:B_LOC, 0:B_LOC])
                xqT2_sb = qn.tile([128, NDC * B_LOC], BF16)
                nc.scalar.copy(out=xqT2_sb, in_=xqT2_psf[:, 0:NDC * B_LOC])
                # per kv: qtT chunks [128 D, (c, h, bl)]
                for kv in range(N_KV):
                    if kv % 2 == 0:
                        qt_ps = scps.tile([128, 512], F32, tag="sc")
                    else:
                        qt_ps = miscps.tile([128, 512], F32, tag="ctp")
                    qp = qt_ps.rearrange("p (c h bl) -> p c h bl", c=NDC, h=HL)
                    for c in range(NDC):
                        nc.tensor.matmul(
                            qt_ps[:, c * 8:(c + 1) * 8],
                            wk_pieces[kv][:, c * 128:(c + 1) * 128],
                            xqT2_sb[:, 8 * kv:8 * (kv + 1)],
                            start=True, stop=True)
                    qall = qtT_all.rearrange("p (bl c hh) -> p bl c hh",
                                             bl=B_LOC, c=NDC)
                    nc.scalar.copy(
                        out=qall[:, :, :, 4 * kv:4 * (kv + 1)],
                        in_=qt_ps[:, 0:256].rearrange(
                            "p (c h bl) -> p bl c h", c=NDC, h=HL))
                if debug:
                    for b in range(B_LOC):
                        nc.sync.dma_start(
                            out=dbg_qtT[:, b * NDC * NH:(b + 1) * NDC * NH],
                            in_=qtT[b])

            # ---------------- Phase B: streaming attention per local batch
            cc2 = [None, None]
            if True:
                xdmas = []
                import os
                _CP = os.environ.get("XTCOPY", "alldve")
                def xt_copy(g, out, in_):
                    # GPSIMD cannot read PSUM; split PSUM->SBUF copies
                    # between DVE and ACT.
                    if _CP == "alldve":
                        nc.vector.tensor_copy(out=out, in_=in_)
                    elif _CP == "2d2a":
                        if g in (1, 3):
                            nc.scalar.copy(out=out, in_=in_)
                        else:
                            nc.vector.tensor_copy(out=out, in_=in_)
                    elif _CP == "split":
                        if g in (1, 3):
                            nc.scalar.copy(out=out[:, 0:512], in_=in_[:, 0:512])
                            nc.vector.tensor_copy(out=out[:, 512:1024],
                                                  in_=in_[:, 512:1024])
                        else:
                            nc.vector.tensor_copy(out=out, in_=in_)
                    else:
                        if g == 1:
                            nc.scalar.copy(out=out, in_=in_)
                        else:
                            nc.vector.tensor_copy(out=out, in_=in_)
                for b in range(B_LOC):
                    ctx_ps = ctxps.tile([128, NDC * NH], F32, tag="ctx")
                    sumT_ps = sumps.tile([NH, 1], F32, tag="sumT")

                    def emit_attn(tt, x_sb, xT_sb):
                        sc_full = scps.tile([128, 512], F32, tag="sc")
                        sc_ps = sc_full[:, 0:NH]
                        for c in range(NDC):
                            nc.tensor.matmul(
                                sc_ps,
                                xT_sb[:, c * 128:(c + 1) * 128],
                                qtT[b][:, c * NH:(c + 1) * NH],
                                start=(c == 0), stop=(c == NDC - 1))
                        at_sb = apool.tile([128, NH], BF16, tag="at")
                        nc.scalar.activation(
                            out=at_sb, in_=sc_ps,
                            func=mybir.ActivationFunctionType.Exp)
                        nc.tensor.matmul(sumT_ps, at_sb, ones_bf,
                                         start=(tt == 0), stop=(tt == NT - 1))
                        bank_start = [None, None]
                        for c in range(NDC):
                            mm = nc.tensor.matmul(
                                ctx_ps[:, c * NH:(c + 1) * NH],
                                x_sb[:, c * 128:(c + 1) * 128],
                                at_sb,
                                start=(tt == 0 and c % 16 == 0),
                                stop=(tt == NT - 1),
                                skip_group_check=True)
                            if tt == 0:
                                if c % 16 == 0:
                                    bank_start[c // 16] = mm
                                else:
                                    add_dep_helper(
                                        mm.ins, bank_start[c // 16].ins,
                                        reason="bank wipe first")

                    pending = None
                    for tt in range(NT):
                        x_sb = xpool.tile([128, DIM], BF16, tag="x",
                                          name=f"x{b}_{tt}")
                        xd = nc.sync.dma_start(
                            out=x_sb, in_=xp[b, tt * 128:(tt + 1) * 128, :])
                        if len(xdmas) == 2 and stage1:
                            add_dep_helper(xd.ins, stage1[-1].ins,
                                           reason="stage before prefetch")
                        if len(xdmas) in (8, 9) and xql_dmas:
                            add_dep_helper(xd.ins, xql_dmas[-1].ins,
                                           reason="xq_loc priority")
                        xdmas.append(xd)
                        xT_sb = xTpool.tile([128, DIM], BF16, tag="xT")
                        for g in range(0 if notrans else 4):
                            tp = tps.tile([128, 1024], BF16, tag="xtp")
                            for k in range(8):
                                c = g * 8 + k
                                nc.tensor.transpose(
                                    tp[:, k * 128:(k + 1) * 128],
                                    x_sb[:, c * 128:(c + 1) * 128], ident)
                            xt_copy(g, xT_sb[:, g * 1024:(g + 1) * 1024], tp)
                        if noattn:
                            continue
                        if pending is not None:
                            emit_attn(*pending)
                        pending = (tt, x_sb, xT_sb)
                    if pending is not None:
                        emit_attn(*pending)
                    # finalize batch b
                    sumT_sb = smallp.tile([NH, 1], F32, tag="ssum")
                    nc.vector.tensor_copy(out=sumT_sb, in_=sumT_ps)
                    recip = smallp.tile([NH, 1], F32, tag="recip")
                    nc.vector.reciprocal(out=recip, in_=sumT_sb)
                    ctxT_sb = ctxsbp.tile([128, NDC * NH], BF16, tag="ctxT")
                    for half in range(2):
                        nc.vector.tensor_copy(
                            out=ctxT_sb[:, half * 512:(half + 1) * 512],
                            in_=ctx_ps[:, half * 512:(half + 1) * 512])
                    ctx_sb = ctxsbp.tile([NH, DIM], BF16, tag="ctxn")
                    for g in range(4):
                        tp2f = tps.tile([128, 1024], BF16, tag="xtp")
                        tp2 = tp2f[0:NH]
                        for k in range(8):
                            c = g * 8 + k
                            nc.tensor.transpose(
                                tp2[:, k * 128:(k + 1) * 128],
                                ctxT_sb[:, c * NH:(c + 1) * NH],
                                ident)
                        nc.vector.tensor_scalar_mul(
                            ctx_sb[:, g * 1024:(g + 1) * 1024], tp2, recip)
                    if debug:
                        nc.sync.dma_start(
                            out=dbg_ctx[:, b * DIM:(b + 1) * DIM], in_=ctx_sb)
                    d = nc.gpsimd.dma_start(out=a2a2_in[b][:, :], in_=ctx_sb)
                    if not nocc:
                        cc2[b] = nc.gpsimd.collective_compute(
                            "AllToAll", mybir.AluOpType.bypass,
                            ins=[a2a2_in[b].opt()], outs=[a2a2_out[b].opt()],
                            replica_groups=rg)
                        add_dep_helper(cc2[b].ins, d.ins,
                                       reason="a2a2 input ready")

                # ---------------- Phase C: output projection per batch slot
                wvT_sb = pCw.tile([128, NDC * HD], BF16)
                dwv = nc.sync.dma_start(out=wvT_sb, in_=wvT[:, :])
                woT_sb = pCw.tile([128, HL * DIM], BF16)
                dwo = nc.sync.dma_start(out=woT_sb, in_=woT[:, :])
                add_dep_helper(dwv.ins, xdmas[NT + 4].ins, reason="late wv")
                add_dep_helper(dwo.ins, xdmas[NT + 8].ins, reason="late wo")
                yT_sb = pCw.tile([128, NDC * B_LOC * NC], F32)
                for b in range(B_LOC):
                    ctxgf = xpool.tile([128, DIM], BF16, tag="x",
                                       name=f"ctxg{b}")
                    ctxg = ctxgf[0:NH]
                    d = nc.scalar.dma_start(out=ctxg, in_=a2a2_out[b][:, :])
                    if cc2[b] is not None:
                        add_dep_helper(d.ins, cc2[b].ins, reason="a2a2 done")
                    ctxgT = pC.tile([128, NDC * NH], BF16, tag="ctxgT")
                    tpg = tps.tile([128, 1024], BF16, tag="xtp")
                    for c in range(NDC):
                        nc.tensor.transpose(
                            tpg[:, c * 32:(c + 1) * 32],
                            ctxg[:, c * 128:(c + 1) * 128],
                            ident[0:NH, 0:NH])
                    nc.vector.tensor_copy(out=ctxgT, in_=tpg)
                    # outT[d, (s,h)] = sum_D wvT[D, d]^T ctxgT[D, (s,h)]
                    op_ps = pCps.tile([HD, NH], F32, tag="ctp")
                    for c in range(NDC):
                        nc.tensor.matmul(op_ps,
                                         wvT_sb[:, c * 128:(c + 1) * 128],
                                         ctxgT[:, c * NH:(c + 1) * NH],
                                         start=(c == 0), stop=(c == NDC - 1))
                    outT = pC.tile([128, NH], BF16, tag="outT")
                    nc.vector.tensor_copy(out=outT[0:HD], in_=op_ps)
                    # yT[j, s] = sum_h sum_d woT[d, (h, jc, j)] * outT[d, (s, h)]
                    ov = outT.rearrange("p (s h) -> p h s", h=HL)
                    y_ps = yps.tile([128, NDC * NC], F32, tag="ctp")
                    for jc in range(NDC):
                        for h in range(HL):
                            nc.tensor.matmul(
                                y_ps[:, jc * NC:(jc + 1) * NC],
                                woT_sb[:, h * DIM + jc * 128:
                                       h * DIM + (jc + 1) * 128],
                                ov[:, h, :],
                                start=(h == 0), stop=(h == HL - 1))
                    yv = yT_sb.rearrange("p (jc b s) -> b p jc s", jc=NDC,
                                         b=B_LOC)
                    nc.vector.tensor_copy(
                        out=yv[b],
                        in_=y_ps.rearrange("p (jc s) -> p jc s", jc=NDC))
                nc.sync.dma_start(out=yT[:, :], in_=yT_sb)

    nc.finalize()
    return nc


_PROGRAM_CACHE = {}


def _prep_inputs(x_pre, wq, wk, wv, wo):
    """Shard + cast + pre-transpose on host. Returns in_maps for 8 cores."""
    xlT_full = np.ascontiguousarray(
        x_pre[:, -1, :].T.astype(NPBF))                    # [4096, 16]
    xlT_full = xlT_full.reshape(NDC, 128, BSZ).transpose(1, 0, 2)  # [128,c,b]
    xlT_flat = np.ascontiguousarray(xlT_full.reshape(128, NDC * BSZ))

    wk_s = (wk * SCALE).astype(NPBF)
    in_maps = []
    for r in range(NC):
        # wqT[p, c, h, o] = wq[512r + h*128 + o, c*128 + p]
        wq_sl = wq[512 * r:512 * (r + 1), :].astype(NPBF)   # [512, 4096] (h,o)xD
        wqT_r = wq_sl.reshape(HL, 128, NDC, 128).transpose(3, 2, 0, 1)
        wqT_r = np.ascontiguousarray(wqT_r.reshape(128, NDC * HL * HD))
        # wvT[p, c, d] = wv[128r + d, c*128 + p]
        wv_sl = wv[128 * r:128 * (r + 1), :].astype(NPBF)   # [128 d, 4096 D]
        wvT_r = wv_sl.reshape(128, NDC, 128).transpose(2, 1, 0)
        wvT_r = np.ascontiguousarray(wvT_r.reshape(128, NDC * HD))
        # woT[p, h, jc, j] = wo[jc*128 + j, 512r + h*128 + p]
        wo_sl = wo[:, 512 * r:512 * (r + 1)].astype(NPBF)   # [4096 j, 512 o]
        woT_r = wo_sl.reshape(NDC, 128, HL, 128).transpose(3, 2, 0, 1)
        woT_r = np.ascontiguousarray(woT_r.reshape(128, HL * DIM))
        in_maps.append({
            "xp": np.ascontiguousarray(x_pre[2 * r:2 * r + 2].astype(NPBF)),
            "xlT": xlT_flat,
            "wqT": wqT_r,
            "wk": np.ascontiguousarray(wk_s),
            "wvT": wvT_r,
            "woT": woT_r,
        })
    return in_maps


def kernel(x_pre, wq, wk, wv, wo, _trace=False, _tmpdir=None, _debug=False):
    x_pre = np.asarray(x_pre, dtype=np.float32)
    wq = np.asarray(wq, dtype=np.float32)
    wk = np.asarray(wk, dtype=np.float32)
    wv = np.asarray(wv, dtype=np.float32)
    wo = np.asarray(wo, dtype=np.float32)

    key = "nc_dbg" if _debug else "nc"
    if key not in _PROGRAM_CACHE:
        _PROGRAM_CACHE[key] = build_program(debug=_debug)
        _PROGRAM_CACHE["nc"] = _PROGRAM_CACHE[key]
    nc = _PROGRAM_CACHE[key]

    in_maps = _prep_inputs(x_pre, wq, wk, wv, wo)

    kwargs = {}
    if _trace:
        kwargs = dict(trace=True, trace_cores=[0])
    if _tmpdir is not None:
        kwargs["tmpdir"] = _tmpdir
    res = run_bass_kernel_spmd(nc, in_maps, core_ids=list(range(NC)), **kwargs)

    y = np.zeros((BSZ, DIM), np.float64)
    for r in range(NC):
        yT_r = np.asarray(res.results[r]["yT"], np.float32)
        yT_r = yT_r.reshape(128, NDC, B_LOC, NC)
        # y[2s+b, jc*128+p] += yT_r[p, jc, b, s]
        y += yT_r.transpose(3, 2, 1, 0).reshape(BSZ, DIM)
    if _debug:
        _PROGRAM_CACHE["dbg"] = res
    if _trace:
        print("HW exec time:", res.exec_time_ns, "ns")
    return y.astype(np.float32).reshape(BSZ, 1, DIM)


# revision 54
# speedup vs baseline: 4.7612x; 1.0075x over previous
"""Bass/Trainium2 kernel for GQA decode attention (fused K-projection form).

Reference computation:
  x = x_pre[:, -1, :]                               # [16, 4096]
  xq = (x @ wq.T) -> [b, 32, 128]
  qt[b,h,:] = xq[b,h,:] @ wk[kv(h)*128:+128, :]     # [b, 32, 4096]
  scores = qt . x_pre / sqrt(128)                   # [b, 32, 2048]
  attn = softmax_t(scores)
  ctx[b,h,:] = sum_t attn[b,h,t] * x_pre[b,t,:]     # [b, 32, 4096]  (lazy-V)
  out[b,h,d] = sum_D ctx[b,h,D] * wv[kv(h)*128+d,D] # [b, 32, 128]
  y = out.flat @ wo.T                               # [16, 4096]

Sharding (8 cores): batch-parallel attention (2 batches/core) +
head-parallel projections (4 heads = 1 kv group/core), exchanged with
AllToAll collectives.  All device data is bf16 (f32 PSUM accumulation);
weights are pre-transposed on the host into the layouts the PE consumes,
and the big matmuls are arranged stationary-heavy (large lhsT, narrow
moving operand) so PE streaming cost is minimized.
"""

import math

import numpy as np
import ml_dtypes

import concourse.bass as bass
import concourse.mybir as mybir
import concourse.tile as tile
from concourse import bacc
from concourse.bass_utils import run_bass_kernel_spmd
from concourse.masks import make_identity
from concourse.tile import add_dep_helper

F32 = mybir.dt.float32
BF16 = mybir.dt.bfloat16
NPBF = ml_dtypes.bfloat16

NC = 8
BSZ = 16
SEQ = 2048
DIM = 4096
NH = 32
HD = 128
B_LOC = 2        # batches per core
HL = 4           # local heads per core (= one kv group)
N_KV = 8
NT = SEQ // 128  # 16 t-tiles per batch
NDC = DIM // 128 # 32 D-chunks
SCALE = 1.0 / math.sqrt(HD)


def build_program(debug=False, nocc=False, noattn=False, notrans=False, nocopy=False):
    nc = bacc.Bacc("TRN2", target_bir_lowering=False, debug=False)

    xp = nc.dram_tensor("xp", [B_LOC, SEQ, DIM], BF16, kind="ExternalInput")
    # xlT[p, c, b] = x_pre[b, -1, c*128+p]
    xlT = nc.dram_tensor("xlT", [128, NDC * BSZ], BF16, kind="ExternalInput")
    # wqT[p, c*512 + h*128 + o] = wq[512r + h*128 + o, c*128 + p]
    wqT = nc.dram_tensor("wqT", [128, NDC * HL * HD], BF16,
                         kind="ExternalInput")
    # wk_s = wk * SCALE  (full, natural [kv*128+d, D])
    wk = nc.dram_tensor("wk", [N_KV * HD, DIM], BF16, kind="ExternalInput")
    # wvT[p, c*128 + d] = wv[128r + d, c*128 + p]
    wvT = nc.dram_tensor("wvT", [128, NDC * HD], BF16, kind="ExternalInput")
    # woT[p, h*4096 + jc*128 + j] = wo[jc*128 + j, 512r + h*128 + p]
    woT = nc.dram_tensor("woT", [128, HL * DIM], BF16, kind="ExternalInput")
    # yT[p, jc*16 + b*8 + s] = y_partial[2s+b, jc*128+p]
    yT = nc.dram_tensor("yT", [128, NDC * B_LOC * NC], BF16,
                        kind="ExternalOutput")
    if debug:
        dbg_xq = nc.dram_tensor("dbg_xq", [128, 64], BF16, kind="ExternalOutput")
        dbg_qt = nc.dram_tensor("dbg_qt", [64, DIM], BF16, kind="ExternalOutput")
        dbg_qtT = nc.dram_tensor("dbg_qtT", [128, B_LOC * NDC * NH],
                                 BF16, kind="ExternalOutput")
        dbg_ctx = nc.dram_tensor("dbg_ctx", [NH, B_LOC * DIM], BF16,
                                 kind="ExternalOutput")
        dbg_out = nc.dram_tensor("dbg_out", [NH, B_LOC * HD], BF16,
                                 kind="ExternalOutput")

    rg = [list(range(NC))]
    vs_engines = None  # round-robin copy engines, set below

    with tile.TileContext(nc) as tc:
        with (
            tc.tile_pool(name="persist", bufs=1) as pers,
            tc.tile_pool(name="dram", bufs=1, space="DRAM") as dram,
            tc.tile_pool(name="xpool", bufs=6) as xpool,
            tc.tile_pool(name="xTpool", bufs=3) as xTpool,
            tc.tile_pool(name="attn", bufs=8) as apool,
            tc.tile_pool(name="small", bufs=2) as smallp,
            tc.tile_pool(name="ctxsb", bufs=1) as ctxsbp,
            tc.tile_pool(name="pC", bufs=1) as pC,
            tc.tile_pool(name="pCw", bufs=1) as pCw,
            tc.tile_pool(name="tps", bufs=3, space="PSUM") as tps,
            tc.tile_pool(name="scps", bufs=1, space="PSUM") as scps,
                                    tc.tile_pool(name="ctxps", bufs=1, space="PSUM") as ctxps,
            tc.tile_pool(name="sumps", bufs=1, space="PSUM") as sumps,
            tc.tile_pool(name="miscps", bufs=1, space="PSUM") as miscps,
        ):
            fps = miscps
            pCps = miscps
            yps = miscps
            ident = pers.tile([128, 128], BF16)
            make_identity(nc, ident)
            ones_bf = pers.tile([128, 1], BF16)
            nc.vector.memset(ones_bf, 1.0)

            a2a1_in = dram.tile([BSZ, 512], BF16)
            a2a1_out = dram.tile([BSZ, 512], BF16)
            a2a2_in = [dram.tile([NC * HL, DIM], BF16, name=f"a2a2i{b}")
                       for b in range(B_LOC)]
            a2a2_out = [dram.tile([NC * HL, DIM], BF16, name=f"a2a2o{b}")
                        for b in range(B_LOC)]

            # ---------------- Phase A: xq (head-sharded) -> tiny AllToAll
            qtT_all = pers.tile([128, B_LOC * NDC * NH], BF16, name="qtTall")
            qtT = [qtT_all[:, b * NDC * NH:(b + 1) * NDC * NH]
                   for b in range(B_LOC)]
            stage1 = []
            with (
                tc.tile_pool(name="pA", bufs=1) as pA,
                tc.tile_pool(name="pAw", bufs=2) as pAw,
            ):
                xlT_sb = pA.tile([128, NDC * BSZ], BF16)
                nc.sync.dma_start(out=xlT_sb, in_=xlT[:, :])
                wq_pieces = []
                for q in range(4):
                    wq_q = pAw.tile([128, 8 * HL * HD], BF16, tag="wqq",
                                    name=f"wqq{q}")
                    nc.sync.dma_start(
                        out=wq_q, in_=wqT[:, q * 4096:(q + 1) * 4096])
                    wq_pieces.append(wq_q)
                # xq[b, o] for the local 512-wide o-slice; one PSUM chain
                xq_psf = scps.tile([128, 512], F32, tag="sc")
                xq_ps = xq_psf[0:BSZ]
                for q in range(4):
                    for k in range(8):
                        c = q * 8 + k
                        nc.tensor.matmul(
                            xq_ps,
                            xlT_sb[:, c * BSZ:(c + 1) * BSZ],
                            wq_pieces[q][:, k * 512:(k + 1) * 512],
                            start=(c == 0), stop=(c == NDC - 1))
                xq_sb = pA.tile([BSZ, 512], BF16)
                nc.scalar.copy(out=xq_sb, in_=xq_ps)
                d = nc.scalar.dma_start(out=a2a1_in[:, :], in_=xq_sb)
                stage1.append(d)

            cc1 = None
            if not nocc:
                cc1 = nc.gpsimd.collective_compute(
                    "AllToAll", mybir.AluOpType.bypass,
                    ins=[a2a1_in.opt()], outs=[a2a1_out.opt()],
                    replica_groups=rg)
                for d in stage1:
                    add_dep_helper(cc1.ins, d.ins, reason="a2a1 input ready")

            # qtT[b][p=D, c*32+h] = sum_d xq[2r+b, h*128+d] * wk_s[h*128+d, c*128+p]
            with (
                tc.tile_pool(name="qn", bufs=1) as qn,
                tc.tile_pool(name="wkp", bufs=8) as wkp,
            ):
                # wk pieces per kv group, streamed (SP queue, after wqT)
                wk_dmas = []
                wk_pieces = []
                for kv in range(N_KV):
                    wkq = wkp.tile([HD, DIM], BF16, tag="wkp",
                                   name=f"wk{kv}")
                    dk = nc.scalar.dma_start(
                        out=wkq, in_=wk[kv * HD:(kv + 1) * HD, :])

                    wk_dmas.append(dk)
                    wk_pieces.append(wkq)
                xq_loc = qn.tile([B_LOC, DIM], BF16)
                xql_dmas = []
                av = a2a1_out.rearrange("(sq bl) o -> bl sq o", sq=NC)
                for bl in range(B_LOC):
                    d = nc.scalar.dma_start(
                        out=xq_loc[bl:bl + 1].rearrange(
                            "p (sq o) -> p sq o", sq=NC),
                        in_=av[bl])
                    xql_dmas.append(d)
                    if cc1 is not None:
                        add_dep_helper(d.ins, cc1.ins, reason="a2a1 done")
                # xqT2[p=d, 2*hg+bl] via PE transposes
                xqT2_psf = miscps.tile([128, 512], BF16, tag="ctp")
                for c in range(NDC):
                    nc.tensor.transpose(
                        xqT2_psf[:, c * 2:(c + 1) * 2],
                        xq_loc[:, c * 128:(c + 1) * 128],
                        ident[0:B_LOC, 0:B_LOC])
                xqT2_sb = qn.tile([128, NDC * B_LOC], BF16)
                nc.scalar.copy(out=xqT2_sb, in_=xqT2_psf[:, 0:NDC * B_LOC])
                # per kv: qtT chunks [128 D, (c, h, bl)]
                for kv in range(N_KV):
                    if kv % 2 == 0:
                        qt_ps = scps.tile([128, 512], F32, tag="sc")
                    else:
                        qt_ps = miscps.tile([128, 512], F32, tag="ctp")
                    qp = qt_ps.rearrange("p (c h bl) -> p c h bl", c=NDC, h=HL)
                    for c in range(NDC):
                        nc.tensor.matmul(
                            qt_ps[:, c * 8:(c + 1) * 8],
                            wk_pieces[kv][:, c * 128:(c + 1) * 128],
                            xqT2_sb[:, 8 * kv:8 * (kv + 1)],
                            start=True, stop=True)
                    qall = qtT_all.rearrange("p (bl c hh) -> p bl c hh",
                                             bl=B_LOC, c=NDC)
                    nc.scalar.copy(
                        out=qall[:, :, :, 4 * kv:4 * (kv + 1)],
                        in_=qt_ps[:, 0:256].rearrange(
                            "p (c h bl) -> p bl c h", c=NDC, h=HL))
                if debug:
                    for b in range(B_LOC):
                        nc.sync.dma_start(
                            out=dbg_qtT[:, b * NDC * NH:(b + 1) * NDC * NH],
                            in_=qtT[b])

            # ---------------- Phase B: streaming attention per local batch
            cc2 = [None, None]
            if True:
                xdmas = []
                import os
                _CP = os.environ.get("XTCOPY", "3d1a")
                def xt_copy(g, out, in_):
                    # GPSIMD cannot read PSUM; split PSUM->SBUF copies
                    # between DVE and ACT.
                    if _CP == "alldve":
                        nc.vector.tensor_copy(out=out, in_=in_)
                    elif _CP == "2d2a":
                        if g in (1, 3):
                            nc.scalar.copy(out=out, in_=in_)
                        else:
                            nc.vector.tensor_copy(out=out, in_=in_)
                    elif _CP == "split":
                        if g in (1, 3):
                            nc.scalar.copy(out=out[:, 0:512], in_=in_[:, 0:512])
                            nc.vector.tensor_copy(out=out[:, 512:1024],
                                                  in_=in_[:, 512:1024])
                        else:
                            nc.vector.tensor_copy(out=out, in_=in_)
                    else:
                        if g == 1:
                            nc.scalar.copy(out=out, in_=in_)
                        else:
                            nc.vector.tensor_copy(out=out, in_=in_)
                for b in range(B_LOC):
                    ctx_ps = ctxps.tile([128, NDC * NH], F32, tag="ctx")
                    sumT_ps = sumps.tile([NH, 1], F32, tag="sumT")

                    def emit_attn(tt, x_sb, xT_sb):
                        sc_full = scps.tile([128, 512], F32, tag="sc")
                        sc_ps = sc_full[:, 0:NH]
                        for c in range(NDC):
                            nc.tensor.matmul(
                                sc_ps,
                                xT_sb[:, c * 128:(c + 1) * 128],
                                qtT[b][:, c * NH:(c + 1) * NH],
                                start=(c == 0), stop=(c == NDC - 1))
                        at_sb = apool.tile([128, NH], BF16, tag="at")
                        nc.scalar.activation(
                            out=at_sb, in_=sc_ps,
                            func=mybir.ActivationFunctionType.Exp)
                        nc.tensor.matmul(sumT_ps, at_sb, ones_bf,
                                         start=(tt == 0), stop=(tt == NT - 1))
                        bank_start = [None, None]
                        for c in range(NDC):
                            mm = nc.tensor.matmul(
                                ctx_ps[:, c * NH:(c + 1) * NH],
                                x_sb[:, c * 128:(c + 1) * 128],
                                at_sb,
                                start=(tt == 0 and c % 16 == 0),
                                stop=(tt == NT - 1),
                                skip_group_check=True)
                            if tt == 0:
                                if c % 16 == 0:
                                    bank_start[c // 16] = mm
                                else:
                                    add_dep_helper(
                                        mm.ins, bank_start[c // 16].ins,
                                        reason="bank wipe first")

                    pending = None
                    for tt in range(NT):
                        x_sb = xpool.tile([128, DIM], BF16, tag="x",
                                          name=f"x{b}_{tt}")
                        xd = nc.sync.dma_start(
                            out=x_sb, in_=xp[b, tt * 128:(tt + 1) * 128, :])
                        if len(xdmas) == 2 and stage1:
                            add_dep_helper(xd.ins, stage1[-1].ins,
                                           reason="stage before prefetch")
                        import os as _os2
                        _WKPIN = int(_os2.environ.get("WKPIN", "0"))
                        if _WKPIN and 3 <= len(xdmas) <= 10:
                            add_dep_helper(xd.ins,
                                           wk_dmas[len(xdmas) - 3].ins,
                                           reason="wk pacing")
                        if len(xdmas) in (8, 9) and xql_dmas:
                            add_dep_helper(xd.ins, xql_dmas[-1].ins,
                                           reason="xq_loc priority")
                        xdmas.append(xd)
                        xT_sb = xTpool.tile([128, DIM], BF16, tag="xT")
                        for g in range(0 if notrans else 4):
                            tp = tps.tile([128, 1024], BF16, tag="xtp")
                            for k in range(8):
                                c = g * 8 + k
                                nc.tensor.transpose(
                                    tp[:, k * 128:(k + 1) * 128],
                                    x_sb[:, c * 128:(c + 1) * 128], ident)
                            xt_copy(g, xT_sb[:, g * 1024:(g + 1) * 1024], tp)
                        if noattn:
                            continue
                        if pending is not None:
                            emit_attn(*pending)
                        pending = (tt, x_sb, xT_sb)
                    if pending is not None:
                        emit_attn(*pending)
                    # finalize batch b
                    sumT_sb = smallp.tile([NH, 1], F32, tag="ssum")
                    nc.vector.tensor_copy(out=sumT_sb, in_=sumT_ps)
                    recip = smallp.tile([NH, 1], F32, tag="recip")
                    nc.vector.reciprocal(out=recip, in_=sumT_sb)
                    ctxT_sb = ctxsbp.tile([128, NDC * NH], BF16, tag="ctxT")
                    for half in range(2):
                        nc.scalar.copy(
                            out=ctxT_sb[:, half * 512:(half + 1) * 512],
                            in_=ctx_ps[:, half * 512:(half + 1) * 512])
                    ctx_sb = ctxsbp.tile([NH, DIM], BF16, tag="ctxn")
                    for g in range(4):
                        tp2f = tps.tile([128, 1024], BF16, tag="xtp")
                        tp2 = tp2f[0:NH]
                        for k in range(8):
                            c = g * 8 + k
                            nc.tensor.transpose(
                                tp2[:, k * 128:(k + 1) * 128],
                                ctxT_sb[:, c * NH:(c + 1) * NH],
                                ident)
                        nc.vector.tensor_scalar_mul(
                            ctx_sb[:, g * 1024:(g + 1) * 1024], tp2, recip)
                    if debug:
                        nc.sync.dma_start(
                            out=dbg_ctx[:, b * DIM:(b + 1) * DIM], in_=ctx_sb)
                    d = nc.gpsimd.dma_start(out=a2a2_in[b][:, :], in_=ctx_sb)
                    if not nocc:
                        cc2[b] = nc.gpsimd.collective_compute(
                            "AllToAll", mybir.AluOpType.bypass,
                            ins=[a2a2_in[b].opt()], outs=[a2a2_out[b].opt()],
                            replica_groups=rg)
                        add_dep_helper(cc2[b].ins, d.ins,
                                       reason="a2a2 input ready")

                # ---------------- Phase C: output projection per batch slot
                wvT_sb = pCw.tile([128, NDC * HD], BF16)
                dwv = nc.sync.dma_start(out=wvT_sb, in_=wvT[:, :])
                woT_sb = pCw.tile([128, HL * DIM], BF16)
                dwo = nc.sync.dma_start(out=woT_sb, in_=woT[:, :])
                add_dep_helper(dwv.ins, xdmas[NT + 4].ins, reason="late wv")
                add_dep_helper(dwo.ins, xdmas[NT + 8].ins, reason="late wo")
                yT_sb = pCw.tile([128, NDC * B_LOC * NC], BF16)
                for b in range(B_LOC):
                    ctxgf = xpool.tile([128, DIM], BF16, tag="x",
                                       name=f"ctxg{b}")
                    ctxg = ctxgf[0:NH]
                    d = nc.scalar.dma_start(out=ctxg, in_=a2a2_out[b][:, :])
                    if cc2[b] is not None:
                        add_dep_helper(d.ins, cc2[b].ins, reason="a2a2 done")
                    ctxgT = pC.tile([128, NDC * NH], BF16, tag="ctxgT")
                    tpg = tps.tile([128, 1024], BF16, tag="xtp")
                    for c in range(NDC):
                        nc.tensor.transpose(
                            tpg[:, c * 32:(c + 1) * 32],
                            ctxg[:, c * 128:(c + 1) * 128],
                            ident[0:NH, 0:NH])
                    nc.vector.tensor_copy(out=ctxgT, in_=tpg)
                    # outT[d, (s,h)] = sum_D wvT[D, d]^T ctxgT[D, (s,h)]
                    op_ps = pCps.tile([HD, NH], F32, tag="ctp")
                    for c in range(NDC):
                        nc.tensor.matmul(op_ps,
                                         wvT_sb[:, c * 128:(c + 1) * 128],
                                         ctxgT[:, c * NH:(c + 1) * NH],
                                         start=(c == 0), stop=(c == NDC - 1))
                    outT = pC.tile([128, NH], BF16, tag="outT")
                    nc.vector.tensor_copy(out=outT[0:HD], in_=op_ps)
                    # yT[j, s] = sum_h sum_d woT[d, (h, jc, j)] * outT[d, (s, h)]
                    ov = outT.rearrange("p (s h) -> p h s", h=HL)
                    y_ps = yps.tile([128, NDC * NC], F32, tag="ctp")
                    for jc in range(NDC):
                        for h in range(HL):
                            nc.tensor.matmul(
                                y_ps[:, jc * NC:(jc + 1) * NC],
                                woT_sb[:, h * DIM + jc * 128:
                                       h * DIM + (jc + 1) * 128],
                                ov[:, h, :],
                                start=(h == 0), stop=(h == HL - 1))
                    yv = yT_sb.rearrange("p (jc b s) -> b p jc s", jc=NDC,
                                         b=B_LOC)
                    nc.vector.tensor_copy(
                        out=yv[b],
                        in_=y_ps.rearrange("p (jc s) -> p jc s", jc=NDC))
                nc.sync.dma_start(out=yT[:, :], in_=yT_sb)

    nc.finalize()
    return nc


_PROGRAM_CACHE = {}


def _prep_inputs(x_pre, wq, wk, wv, wo):
    """Shard + cast + pre-transpose on host. Returns in_maps for 8 cores."""
    xlT_full = np.ascontiguousarray(
        x_pre[:, -1, :].T.astype(NPBF))                    # [4096, 16]
    xlT_full = xlT_full.reshape(NDC, 128, BSZ).transpose(1, 0, 2)  # [128,c,b]
    xlT_flat = np.ascontiguousarray(xlT_full.reshape(128, NDC * BSZ))

    wk_s = (wk * SCALE).astype(NPBF)
    in_maps = []
    for r in range(NC):
        # wqT[p, c, h, o] = wq[512r + h*128 + o, c*128 + p]
        wq_sl = wq[512 * r:512 * (r + 1), :].astype(NPBF)   # [512, 4096] (h,o)xD
        wqT_r = wq_sl.reshape(HL, 128, NDC, 128).transpose(3, 2, 0, 1)
        wqT_r = np.ascontiguousarray(wqT_r.reshape(128, NDC * HL * HD))
        # wvT[p, c, d] = wv[128r + d, c*128 + p]
        wv_sl = wv[128 * r:128 * (r + 1), :].astype(NPBF)   # [128 d, 4096 D]
        wvT_r = wv_sl.reshape(128, NDC, 128).transpose(2, 1, 0)
        wvT_r = np.ascontiguousarray(wvT_r.reshape(128, NDC * HD))
        # woT[p, h, jc, j] = wo[jc*128 + j, 512r + h*128 + p]
        wo_sl = wo[:, 512 * r:512 * (r + 1)].astype(NPBF)   # [4096 j, 512 o]
        woT_r = wo_sl.reshape(NDC, 128, HL, 128).transpose(3, 2, 0, 1)
        woT_r = np.ascontiguousarray(woT_r.reshape(128, HL * DIM))
        in_maps.append({
            "xp": np.ascontiguousarray(x_pre[2 * r:2 * r + 2].astype(NPBF)),
            "xlT": xlT_flat,
            "wqT": wqT_r,
            "wk": np.ascontiguousarray(wk_s),
            "wvT": wvT_r,
            "woT": woT_r,
        })
    return in_maps


def kernel(x_pre, wq, wk, wv, wo, _trace=False, _tmpdir=None, _debug=False):
    x_pre = np.asarray(x_pre, dtype=np.float32)
    wq = np.asarray(wq, dtype=np.float32)
    wk = np.asarray(wk, dtype=np.float32)
    wv = np.asarray(wv, dtype=np.float32)
    wo = np.asarray(wo, dtype=np.float32)

    key = "nc_dbg" if _debug else "nc"
    if key not in _PROGRAM_CACHE:
        _PROGRAM_CACHE[key] = build_program(debug=_debug)
        _PROGRAM_CACHE["nc"] = _PROGRAM_CACHE[key]
    nc = _PROGRAM_CACHE[key]

    in_maps = _prep_inputs(x_pre, wq, wk, wv, wo)

    kwargs = {}
    if _trace:
        kwargs = dict(trace=True, trace_cores=[0])
    if _tmpdir is not None:
        kwargs["tmpdir"] = _tmpdir
    res = run_bass_kernel_spmd(nc, in_maps, core_ids=list(range(NC)), **kwargs)

    y = np.zeros((BSZ, DIM), np.float64)
    for r in range(NC):
        yT_r = np.asarray(res.results[r]["yT"], np.float32)
        yT_r = yT_r.reshape(128, NDC, B_LOC, NC)
        # y[2s+b, jc*128+p] += yT_r[p, jc, b, s]
        y += yT_r.transpose(3, 2, 1, 0).reshape(BSZ, DIM)
    if _debug:
        _PROGRAM_CACHE["dbg"] = res
    if _trace:
        print("HW exec time:", res.exec_time_ns, "ns")
    return y.astype(np.float32).reshape(BSZ, 1, DIM)


# revision 59
# speedup vs baseline: 4.7999x; 1.0081x over previous
"""Bass/Trainium2 kernel for GQA decode attention (fused K-projection form).

Reference computation:
  x = x_pre[:, -1, :]                               # [16, 4096]
  xq = (x @ wq.T) -> [b, 32, 128]
  qt[b,h,:] = xq[b,h,:] @ wk[kv(h)*128:+128, :]     # [b, 32, 4096]
  scores = qt . x_pre / sqrt(128)                   # [b, 32, 2048]
  attn = softmax_t(scores)
  ctx[b,h,:] = sum_t attn[b,h,t] * x_pre[b,t,:]     # [b, 32, 4096]  (lazy-V)
  out[b,h,d] = sum_D ctx[b,h,D] * wv[kv(h)*128+d,D] # [b, 32, 128]
  y = out.flat @ wo.T                               # [16, 4096]

Sharding (8 cores): batch-parallel attention (2 batches/core) +
head-parallel projections (4 heads = 1 kv group/core), exchanged with
AllToAll collectives.  All device data is bf16 (f32 PSUM accumulation);
weights are pre-transposed on the host into the layouts the PE consumes,
and the big matmuls are arranged stationary-heavy (large lhsT, narrow
moving operand) so PE streaming cost is minimized.
"""

import math

import numpy as np
import ml_dtypes

import concourse.bass as bass
import concourse.mybir as mybir
import concourse.tile as tile
from concourse import bacc
from concourse.bass_utils import run_bass_kernel_spmd
from concourse.masks import make_identity
from concourse.tile import add_dep_helper

F32 = mybir.dt.float32
BF16 = mybir.dt.bfloat16
NPBF = ml_dtypes.bfloat16

NC = 8
BSZ = 16
SEQ = 2048
DIM = 4096
NH = 32
HD = 128
B_LOC = 2        # batches per core
HL = 4           # local heads per core (= one kv group)
N_KV = 8
NT = SEQ // 128  # 16 t-tiles per batch
NDC = DIM // 128 # 32 D-chunks
SCALE = 1.0 / math.sqrt(HD)


def build_program(debug=False, nocc=False, noattn=False, notrans=False, nocopy=False):
    nc = bacc.Bacc("TRN2", target_bir_lowering=False, debug=False)

    xp = nc.dram_tensor("xp", [B_LOC, SEQ, DIM], BF16, kind="ExternalInput")
    # xlT[p, c, b] = x_pre[b, -1, c*128+p]
    xlT = nc.dram_tensor("xlT", [128, NDC * BSZ], BF16, kind="ExternalInput")
    # wqT[p, c*512 + h*128 + o] = wq[512r + h*128 + o, c*128 + p]
    wqT = nc.dram_tensor("wqT", [128, NDC * HL * HD], BF16,
                         kind="ExternalInput")
    # wk_s = wk * SCALE  (full, natural [kv*128+d, D])
    wk = nc.dram_tensor("wk", [N_KV * HD, DIM], BF16, kind="ExternalInput")
    # wvT[p, c*128 + d] = wv[128r + d, c*128 + p]
    wvT = nc.dram_tensor("wvT", [128, NDC * HD], BF16, kind="ExternalInput")
    # woT[p, h*4096 + jc*128 + j] = wo[jc*128 + j, 512r + h*128 + p]
    woT = nc.dram_tensor("woT", [128, HL * DIM], BF16, kind="ExternalInput")
    # yT[p, jc*16 + b*8 + s] = y_partial[2s+b, jc*128+p]
    yT = nc.dram_tensor("yT", [128, NDC * B_LOC * NC], BF16,
                        kind="ExternalOutput")
    if debug:
        dbg_xq = nc.dram_tensor("dbg_xq", [128, 64], BF16, kind="ExternalOutput")
        dbg_qt = nc.dram_tensor("dbg_qt", [64, DIM], BF16, kind="ExternalOutput")
        dbg_qtT = nc.dram_tensor("dbg_qtT", [128, B_LOC * NDC * NH],
                                 BF16, kind="ExternalOutput")
        dbg_ctx = nc.dram_tensor("dbg_ctx", [NH, B_LOC * DIM], BF16,
                                 kind="ExternalOutput")
        dbg_out = nc.dram_tensor("dbg_out", [NH, B_LOC * HD], BF16,
                                 kind="ExternalOutput")

    rg = [list(range(NC))]
    vs_engines = None  # round-robin copy engines, set below

    with tile.TileContext(nc) as tc:
        with (
            tc.tile_pool(name="persist", bufs=1) as pers,
            tc.tile_pool(name="dram", bufs=1, space="DRAM") as dram,
            tc.tile_pool(name="xpool", bufs=6) as xpool,
            tc.tile_pool(name="xTpool", bufs=3) as xTpool,
            tc.tile_pool(name="attn", bufs=8) as apool,
            tc.tile_pool(name="small", bufs=2) as smallp,
            tc.tile_pool(name="ctxsb", bufs=1) as ctxsbp,
            tc.tile_pool(name="pC", bufs=1) as pC,
            tc.tile_pool(name="pCw", bufs=1) as pCw,
            tc.tile_pool(name="tps", bufs=3, space="PSUM") as tps,
            tc.tile_pool(name="scps", bufs=1, space="PSUM") as scps,
                                    tc.tile_pool(name="ctxps", bufs=1, space="PSUM") as ctxps,
            tc.tile_pool(name="sumps", bufs=1, space="PSUM") as sumps,
            tc.tile_pool(name="miscps", bufs=1, space="PSUM") as miscps,
        ):
            fps = miscps
            pCps = miscps
            yps = miscps
            ident = pers.tile([128, 128], BF16)
            make_identity(nc, ident)
            ones_bf = pers.tile([128, 1], BF16)
            nc.vector.memset(ones_bf, 1.0)

            a2a1_in = dram.tile([BSZ, 512], BF16)
            a2a1_out = dram.tile([BSZ, 512], BF16)
            a2a2_in = [dram.tile([NC * HL, DIM], BF16, name=f"a2a2i{b}")
                       for b in range(B_LOC)]
            a2a2_out = [dram.tile([NC * HL, DIM], BF16, name=f"a2a2o{b}")
                        for b in range(B_LOC)]

            # ---------------- Phase A: xq (head-sharded) -> tiny AllToAll
            qtT_all = pers.tile([128, B_LOC * NDC * NH], BF16, name="qtTall")
            qtT = [qtT_all[:, b * NDC * NH:(b + 1) * NDC * NH]
                   for b in range(B_LOC)]
            stage1 = []
            with (
                tc.tile_pool(name="pA", bufs=1) as pA,
                tc.tile_pool(name="pAw", bufs=2) as pAw,
            ):
                xlT_sb = pA.tile([128, NDC * BSZ], BF16)
                nc.sync.dma_start(out=xlT_sb, in_=xlT[:, :])
                wq_pieces = []
                for q in range(4):
                    wq_q = pAw.tile([128, 8 * HL * HD], BF16, tag="wqq",
                                    name=f"wqq{q}")
                    nc.sync.dma_start(
                        out=wq_q, in_=wqT[:, q * 4096:(q + 1) * 4096])
                    wq_pieces.append(wq_q)
                # xq[b, o] for the local 512-wide o-slice; one PSUM chain
                xq_psf = scps.tile([128, 512], F32, tag="sc")
                xq_ps = xq_psf[0:BSZ]
                for q in range(4):
                    for k in range(8):
                        c = q * 8 + k
                        nc.tensor.matmul(
                            xq_ps,
                            xlT_sb[:, c * BSZ:(c + 1) * BSZ],
                            wq_pieces[q][:, k * 512:(k + 1) * 512],
                            start=(c == 0), stop=(c == NDC - 1))
                xq_sb = pA.tile([BSZ, 512], BF16)
                nc.scalar.copy(out=xq_sb, in_=xq_ps)
                d = nc.scalar.dma_start(out=a2a1_in[:, :], in_=xq_sb)
                stage1.append(d)

            cc1 = None
            if not nocc:
                cc1 = nc.gpsimd.collective_compute(
                    "AllToAll", mybir.AluOpType.bypass,
                    ins=[a2a1_in.opt()], outs=[a2a1_out.opt()],
                    replica_groups=rg)
                for d in stage1:
                    add_dep_helper(cc1.ins, d.ins, reason="a2a1 input ready")

            # qtT[b][p=D, c*32+h] = sum_d xq[2r+b, h*128+d] * wk_s[h*128+d, c*128+p]
            with (
                tc.tile_pool(name="qn", bufs=1) as qn,
                tc.tile_pool(name="wkp", bufs=8) as wkp,
            ):
                # wk pieces per kv group, streamed (SP queue, after wqT)
                wk_dmas = []
                wk_pieces = []
                for kv in range(N_KV):
                    wkq = wkp.tile([HD, DIM], BF16, tag="wkp",
                                   name=f"wk{kv}")
                    dk = nc.scalar.dma_start(
                        out=wkq, in_=wk[kv * HD:(kv + 1) * HD, :])

                    wk_dmas.append(dk)
                    wk_pieces.append(wkq)
                xq_loc = qn.tile([B_LOC, DIM], BF16)
                xql_dmas = []
                av = a2a1_out.rearrange("(sq bl) o -> bl sq o", sq=NC)
                for bl in range(B_LOC):
                    d = nc.scalar.dma_start(
                        out=xq_loc[bl:bl + 1].rearrange(
                            "p (sq o) -> p sq o", sq=NC),
                        in_=av[bl])
                    xql_dmas.append(d)
                    if cc1 is not None:
                        add_dep_helper(d.ins, cc1.ins, reason="a2a1 done")
                # xqT2[p=d, 2*hg+bl] via PE transposes
                xqT2_psf = miscps.tile([128, 512], BF16, tag="ctp")
                for c in range(NDC):
                    nc.tensor.transpose(
                        xqT2_psf[:, c * 2:(c + 1) * 2],
                        xq_loc[:, c * 128:(c + 1) * 128],
                        ident[0:B_LOC, 0:B_LOC])
                xqT2_sb = qn.tile([128, NDC * B_LOC], BF16)
                nc.scalar.copy(out=xqT2_sb, in_=xqT2_psf[:, 0:NDC * B_LOC])
                # per kv: qtT chunks [128 D, (c, h, bl)]
                for kv in range(N_KV):
                    if kv % 2 == 0:
                        qt_ps = scps.tile([128, 512], F32, tag="sc")
                    else:
                        qt_ps = miscps.tile([128, 512], F32, tag="ctp")
                    qp = qt_ps.rearrange("p (c h bl) -> p c h bl", c=NDC, h=HL)
                    for c in range(NDC):
                        nc.tensor.matmul(
                            qt_ps[:, c * 8:(c + 1) * 8],
                            wk_pieces[kv][:, c * 128:(c + 1) * 128],
                            xqT2_sb[:, 8 * kv:8 * (kv + 1)],
                            start=True, stop=True)
                    qall = qtT_all.rearrange("p (bl c hh) -> p bl c hh",
                                             bl=B_LOC, c=NDC)
                    nc.scalar.copy(
                        out=qall[:, :, :, 4 * kv:4 * (kv + 1)],
                        in_=qt_ps[:, 0:256].rearrange(
                            "p (c h bl) -> p bl c h", c=NDC, h=HL))
                if debug:
                    for b in range(B_LOC):
                        nc.sync.dma_start(
                            out=dbg_qtT[:, b * NDC * NH:(b + 1) * NDC * NH],
                            in_=qtT[b])

            # ---------------- Phase B: streaming attention per local batch
            cc2 = [None, None]
            if True:
                xdmas = []
                import os
                _CP = os.environ.get("XTCOPY", "3d1a")
                def xt_copy(g, out, in_):
                    # GPSIMD cannot read PSUM; split PSUM->SBUF copies
                    # between DVE and ACT.
                    if _CP == "alldve":
                        nc.vector.tensor_copy(out=out, in_=in_)
                    elif _CP == "2d2a":
                        if g in (1, 3):
                            nc.scalar.copy(out=out, in_=in_)
                        else:
                            nc.vector.tensor_copy(out=out, in_=in_)
                    elif _CP == "split":
                        if g in (1, 3):
                            nc.scalar.copy(out=out[:, 0:512], in_=in_[:, 0:512])
                            nc.vector.tensor_copy(out=out[:, 512:1024],
                                                  in_=in_[:, 512:1024])
                        else:
                            nc.vector.tensor_copy(out=out, in_=in_)
                    else:
                        if g == 1:
                            nc.scalar.copy(out=out, in_=in_)
                        else:
                            nc.vector.tensor_copy(out=out, in_=in_)
                fin_state = {}

                def finalize_part1(bb, ctx_ps_b, sumT_ps_b):
                    sumT_sb = smallp.tile([NH, 1], F32, tag="ssum")
                    nc.vector.tensor_copy(out=sumT_sb, in_=sumT_ps_b)
                    recip = smallp.tile([NH, 1], F32, tag="recip")
                    nc.vector.reciprocal(out=recip, in_=sumT_sb)
                    ctxT_sb = ctxsbp.tile([128, NDC * NH], BF16, tag="ctxT")
                    for half in range(2):
                        nc.scalar.copy(
                            out=ctxT_sb[:, half * 512:(half + 1) * 512],
                            in_=ctx_ps_b[:, half * 512:(half + 1) * 512])
                    fin_state[bb] = (ctxT_sb, recip)

                def finalize_part2(bb):
                    ctxT_sb, recip = fin_state.pop(bb)
                    ctx_sb = ctxsbp.tile([NH, DIM], BF16, tag="ctxn")
                    for g in range(4):
                        tp2f = tps.tile([128, 1024], BF16, tag="xtp")
                        tp2 = tp2f[0:NH]
                        for k in range(8):
                            c = g * 8 + k
                            nc.tensor.transpose(
                                tp2[:, k * 128:(k + 1) * 128],
                                ctxT_sb[:, c * NH:(c + 1) * NH],
                                ident)
                        nc.vector.tensor_scalar_mul(
                            ctx_sb[:, g * 1024:(g + 1) * 1024], tp2, recip)
                    if debug:
                        nc.sync.dma_start(
                            out=dbg_ctx[:, bb * DIM:(bb + 1) * DIM],
                            in_=ctx_sb)
                    d = nc.gpsimd.dma_start(out=a2a2_in[bb][:, :], in_=ctx_sb)
                    if not nocc:
                        cc2[bb] = nc.gpsimd.collective_compute(
                            "AllToAll", mybir.AluOpType.bypass,
                            ins=[a2a2_in[bb].opt()], outs=[a2a2_out[bb].opt()],
                            replica_groups=rg)
                        add_dep_helper(cc2[bb].ins, d.ins,
                                       reason="a2a2 input ready")

                for b in range(B_LOC):
                    ctx_ps = ctxps.tile([128, NDC * NH], F32, tag="ctx")
                    sumT_ps = sumps.tile([NH, 1], F32, tag="sumT")

                    def emit_attn(tt, x_sb, xT_sb):
                        sc_full = scps.tile([128, 512], F32, tag="sc")
                        sc_ps = sc_full[:, 0:NH]
                        for c in range(NDC):
                            nc.tensor.matmul(
                                sc_ps,
                                xT_sb[:, c * 128:(c + 1) * 128],
                                qtT[b][:, c * NH:(c + 1) * NH],
                                start=(c == 0), stop=(c == NDC - 1))
                        at_sb = apool.tile([128, NH], BF16, tag="at")
                        nc.scalar.activation(
                            out=at_sb, in_=sc_ps,
                            func=mybir.ActivationFunctionType.Exp)
                        nc.tensor.matmul(sumT_ps, at_sb, ones_bf,
                                         start=(tt == 0), stop=(tt == NT - 1))
                        bank_start = [None, None]
                        for c in range(NDC):
                            mm = nc.tensor.matmul(
                                ctx_ps[:, c * NH:(c + 1) * NH],
                                x_sb[:, c * 128:(c + 1) * 128],
                                at_sb,
                                start=(tt == 0 and c % 16 == 0),
                                stop=(tt == NT - 1),
                                skip_group_check=True)
                            if tt == 0:
                                if c % 16 == 0:
                                    bank_start[c // 16] = mm
                                else:
                                    add_dep_helper(
                                        mm.ins, bank_start[c // 16].ins,
                                        reason="bank wipe first")

                    pending = None
                    for tt in range(NT):
                        x_sb = xpool.tile([128, DIM], BF16, tag="x",
                                          name=f"x{b}_{tt}")
                        xd = nc.sync.dma_start(
                            out=x_sb, in_=xp[b, tt * 128:(tt + 1) * 128, :])
                        if len(xdmas) == 2 and stage1:
                            add_dep_helper(xd.ins, stage1[-1].ins,
                                           reason="stage before prefetch")
                        import os as _os2
                        _WKPIN = int(_os2.environ.get("WKPIN", "0"))
                        if _WKPIN and 3 <= len(xdmas) <= 10:
                            add_dep_helper(xd.ins,
                                           wk_dmas[len(xdmas) - 3].ins,
                                           reason="wk pacing")
                        if len(xdmas) in (8, 9) and xql_dmas:
                            add_dep_helper(xd.ins, xql_dmas[-1].ins,
                                           reason="xq_loc priority")
                        xdmas.append(xd)
                        xT_sb = xTpool.tile([128, DIM], BF16, tag="xT")
                        for g in range(0 if notrans else 4):
                            tp = tps.tile([128, 1024], BF16, tag="xtp")
                            for k in range(8):
                                c = g * 8 + k
                                nc.tensor.transpose(
                                    tp[:, k * 128:(k + 1) * 128],
                                    x_sb[:, c * 128:(c + 1) * 128], ident)
                            xt_copy(g, xT_sb[:, g * 1024:(g + 1) * 1024], tp)
                        if noattn:
                            continue
                        if pending is not None:
                            emit_attn(*pending)
                        pending = (tt, x_sb, xT_sb)
                    if pending is not None:
                        emit_attn(*pending)
                    # finalize batch b
                    sumT_sb = smallp.tile([NH, 1], F32, tag="ssum")
                    nc.vector.tensor_copy(out=sumT_sb, in_=sumT_ps)
                    recip = smallp.tile([NH, 1], F32, tag="recip")
                    nc.vector.reciprocal(out=recip, in_=sumT_sb)
                    ctxT_sb = ctxsbp.tile([128, NDC * NH], BF16, tag="ctxT")
                    for half in range(2):
                        nc.scalar.copy(
                            out=ctxT_sb[:, half * 512:(half + 1) * 512],
                            in_=ctx_ps[:, half * 512:(half + 1) * 512])
                    ctx_sb = ctxsbp.tile([NH, DIM], BF16, tag="ctxn")
                    for g in range(4):
                        tp2f = tps.tile([128, 1024], BF16, tag="xtp")
                        tp2 = tp2f[0:NH]
                        for k in range(8):
                            c = g * 8 + k
                            nc.tensor.transpose(
                                tp2[:, k * 128:(k + 1) * 128],
                                ctxT_sb[:, c * NH:(c + 1) * NH],
                                ident)
                        nc.vector.tensor_scalar_mul(
                            ctx_sb[:, g * 1024:(g + 1) * 1024], tp2, recip)
                    if debug:
                        nc.sync.dma_start(
                            out=dbg_ctx[:, b * DIM:(b + 1) * DIM], in_=ctx_sb)
                    d = nc.gpsimd.dma_start(out=a2a2_in[b][:, :], in_=ctx_sb)
                    if not nocc:
                        cc2[b] = nc.gpsimd.collective_compute(
                            "AllToAll", mybir.AluOpType.bypass,
                            ins=[a2a2_in[b].opt()], outs=[a2a2_out[b].opt()],
                            replica_groups=rg)
                        add_dep_helper(cc2[b].ins, d.ins,
                                       reason="a2a2 input ready")

                # ---------------- Phase C: output projection per batch slot
                wvT_sb = pCw.tile([128, NDC * HD], BF16)
                dwv = nc.sync.dma_start(out=wvT_sb, in_=wvT[:, :])
                woT_sb = pCw.tile([128, HL * DIM], BF16)
                dwo = nc.sync.dma_start(out=woT_sb, in_=woT[:, :])
                add_dep_helper(dwv.ins, xdmas[NT + 4].ins, reason="late wv")
                add_dep_helper(dwo.ins, xdmas[NT + 8].ins, reason="late wo")
                yT_sb = pCw.tile([128, NDC * B_LOC * NC], BF16)
                for b in range(B_LOC):
                    ctxgf = xpool.tile([128, DIM], BF16, tag="x",
                                       name=f"ctxg{b}")
                    ctxg = ctxgf[0:NH]
                    d = nc.scalar.dma_start(out=ctxg, in_=a2a2_out[b][:, :])
                    if cc2[b] is not None:
                        add_dep_helper(d.ins, cc2[b].ins, reason="a2a2 done")
                    ctxgT = pC.tile([128, NDC * NH], BF16, tag="ctxgT")
                    tpg = tps.tile([128, 1024], BF16, tag="xtp")
                    for c in range(NDC):
                        nc.tensor.transpose(
                            tpg[:, c * 32:(c + 1) * 32],
                            ctxg[:, c * 128:(c + 1) * 128],
                            ident[0:NH, 0:NH])
                    nc.vector.tensor_copy(out=ctxgT, in_=tpg)
                    # outT[d, (s,h)] = sum_D wvT[D, d]^T ctxgT[D, (s,h)]
                    op_ps = pCps.tile([HD, NH], F32, tag="ctp")
                    for c in range(NDC):
                        nc.tensor.matmul(op_ps,
                                         wvT_sb[:, c * 128:(c + 1) * 128],
                                         ctxgT[:, c * NH:(c + 1) * NH],
                                         start=(c == 0), stop=(c == NDC - 1))
                    outT = pC.tile([128, NH], BF16, tag="outT")
                    nc.vector.tensor_copy(out=outT[0:HD], in_=op_ps)
                    # yT[j, s] = sum_h sum_d woT[d, (h, jc, j)] * outT[d, (s, h)]
                    ov = outT.rearrange("p (s h) -> p h s", h=HL)
                    y_ps = yps.tile([128, NDC * NC], F32, tag="ctp")
                    for jc in range(NDC):
                        for h in range(HL):
                            nc.tensor.matmul(
                                y_ps[:, jc * NC:(jc + 1) * NC],
                                woT_sb[:, h * DIM + jc * 128:
                                       h * DIM + (jc + 1) * 128],
                                ov[:, h, :],
                                start=(h == 0), stop=(h == HL - 1))
                    yv = yT_sb.rearrange("p (jc b s) -> b p jc s", jc=NDC,
                                         b=B_LOC)
                    nc.vector.tensor_copy(
                        out=yv[b],
                        in_=y_ps.rearrange("p (jc s) -> p jc s", jc=NDC))
                nc.sync.dma_start(out=yT[:, :], in_=yT_sb)

    nc.finalize()
    return nc


_PROGRAM_CACHE = {}


def _prep_inputs(x_pre, wq, wk, wv, wo):
    """Shard + cast + pre-transpose on host. Returns in_maps for 8 cores."""
    xlT_full = np.ascontiguousarray(
        x_pre[:, -1, :].T.astype(NPBF))                    # [4096, 16]
    xlT_full = xlT_full.reshape(NDC, 128, BSZ).transpose(1, 0, 2)  # [128,c,b]
    xlT_flat = np.ascontiguousarray(xlT_full.reshape(128, NDC * BSZ))

    wk_s = (wk * SCALE).astype(NPBF)
    in_maps = []
    for r in range(NC):
        # wqT[p, c, h, o] = wq[512r + h*128 + o, c*128 + p]
        wq_sl = wq[512 * r:512 * (r + 1), :].astype(NPBF)   # [512, 4096] (h,o)xD
        wqT_r = wq_sl.reshape(HL, 128, NDC, 128).transpose(3, 2, 0, 1)
        wqT_r = np.ascontiguousarray(wqT_r.reshape(128, NDC * HL * HD))
        # wvT[p, c, d] = wv[128r + d, c*128 + p]
        wv_sl = wv[128 * r:128 * (r + 1), :].astype(NPBF)   # [128 d, 4096 D]
        wvT_r = wv_sl.reshape(128, NDC, 128).transpose(2, 1, 0)
        wvT_r = np.ascontiguousarray(wvT_r.reshape(128, NDC * HD))
        # woT[p, h, jc, j] = wo[jc*128 + j, 512r + h*128 + p]
        wo_sl = wo[:, 512 * r:512 * (r + 1)].astype(NPBF)   # [4096 j, 512 o]
        woT_r = wo_sl.reshape(NDC, 128, HL, 128).transpose(3, 2, 0, 1)
        woT_r = np.ascontiguousarray(woT_r.reshape(128, HL * DIM))
        in_maps.append({
            "xp": np.ascontiguousarray(x_pre[2 * r:2 * r + 2].astype(NPBF)),
            "xlT": xlT_flat,
            "wqT": wqT_r,
            "wk": np.ascontiguousarray(wk_s),
            "wvT": wvT_r,
            "woT": woT_r,
        })
    return in_maps


def kernel(x_pre, wq, wk, wv, wo, _trace=False, _tmpdir=None, _debug=False):
    x_pre = np.asarray(x_pre, dtype=np.float32)
    wq = np.asarray(wq, dtype=np.float32)
    wk = np.asarray(wk, dtype=np.float32)
    wv = np.asarray(wv, dtype=np.float32)
    wo = np.asarray(wo, dtype=np.float32)

    key = "nc_dbg" if _debug else "nc"
    if key not in _PROGRAM_CACHE:
        _PROGRAM_CACHE[key] = build_program(debug=_debug)
        _PROGRAM_CACHE["nc"] = _PROGRAM_CACHE[key]
    nc = _PROGRAM_CACHE[key]

    in_maps = _prep_inputs(x_pre, wq, wk, wv, wo)

    kwargs = {}
    if _trace:
        kwargs = dict(trace=True, trace_cores=[0])
    if _tmpdir is not None:
        kwargs["tmpdir"] = _tmpdir
    res = run_bass_kernel_spmd(nc, in_maps, core_ids=list(range(NC)), **kwargs)

    y = np.zeros((BSZ, DIM), np.float64)
    for r in range(NC):
        yT_r = np.asarray(res.results[r]["yT"], np.float32)
        yT_r = yT_r.reshape(128, NDC, B_LOC, NC)
        # y[2s+b, jc*128+p] += yT_r[p, jc, b, s]
        y += yT_r.transpose(3, 2, 1, 0).reshape(BSZ, DIM)
    if _debug:
        _PROGRAM_CACHE["dbg"] = res
    if _trace:
        print("HW exec time:", res.exec_time_ns, "ns")
    return y.astype(np.float32).reshape(BSZ, 1, DIM)


# revision 62
# speedup vs baseline: 4.8150x; 1.0032x over previous
"""Bass/Trainium2 kernel for GQA decode attention (fused K-projection form).

Reference computation:
  x = x_pre[:, -1, :]                               # [16, 4096]
  xq = (x @ wq.T) -> [b, 32, 128]
  qt[b,h,:] = xq[b,h,:] @ wk[kv(h)*128:+128, :]     # [b, 32, 4096]
  scores = qt . x_pre / sqrt(128)                   # [b, 32, 2048]
  attn = softmax_t(scores)
  ctx[b,h,:] = sum_t attn[b,h,t] * x_pre[b,t,:]     # [b, 32, 4096]  (lazy-V)
  out[b,h,d] = sum_D ctx[b,h,D] * wv[kv(h)*128+d,D] # [b, 32, 128]
  y = out.flat @ wo.T                               # [16, 4096]

Sharding (8 cores): batch-parallel attention (2 batches/core) +
head-parallel projections (4 heads = 1 kv group/core), exchanged with
AllToAll collectives.  All device data is bf16 (f32 PSUM accumulation);
weights are pre-transposed on the host into the layouts the PE consumes,
and the big matmuls are arranged stationary-heavy (large lhsT, narrow
moving operand) so PE streaming cost is minimized.
"""

import math

import numpy as np
import ml_dtypes

import concourse.bass as bass
import concourse.mybir as mybir
import concourse.tile as tile
from concourse import bacc
from concourse.bass_utils import run_bass_kernel_spmd
from concourse.masks import make_identity
from concourse.tile import add_dep_helper

F32 = mybir.dt.float32
BF16 = mybir.dt.bfloat16
NPBF = ml_dtypes.bfloat16

NC = 8
BSZ = 16
SEQ = 2048
DIM = 4096
NH = 32
HD = 128
B_LOC = 2        # batches per core
HL = 4           # local heads per core (= one kv group)
N_KV = 8
NT = SEQ // 128  # 16 t-tiles per batch
NDC = DIM // 128 # 32 D-chunks
SCALE = 1.0 / math.sqrt(HD)


def build_program(debug=False, nocc=False, noattn=False, notrans=False, nocopy=False):
    nc = bacc.Bacc("TRN2", target_bir_lowering=False, debug=False)

    xp = nc.dram_tensor("xp", [B_LOC, SEQ, DIM], BF16, kind="ExternalInput")
    # xlT[p, c, b] = x_pre[b, -1, c*128+p]
    xlT = nc.dram_tensor("xlT", [128, NDC * BSZ], BF16, kind="ExternalInput")
    # wqT[p, c*512 + h*128 + o] = wq[512r + h*128 + o, c*128 + p]
    wqT = nc.dram_tensor("wqT", [128, NDC * HL * HD], BF16,
                         kind="ExternalInput")
    # wk_s = wk * SCALE  (full, natural [kv*128+d, D])
    wk = nc.dram_tensor("wk", [N_KV * HD, DIM], BF16, kind="ExternalInput")
    # wvT[p, c*128 + d] = wv[128r + d, c*128 + p]
    wvT = nc.dram_tensor("wvT", [128, NDC * HD], BF16, kind="ExternalInput")
    # woT[p, h*4096 + jc*128 + j] = wo[jc*128 + j, 512r + h*128 + p]
    woT = nc.dram_tensor("woT", [128, HL * DIM], BF16, kind="ExternalInput")
    # yT[p, b*256 + jc*8 + s] = y_partial[2s+b, jc*128+p]
    yT = nc.dram_tensor("yT", [128, B_LOC * NDC * NC], BF16,
                        kind="ExternalOutput")
    if debug:
        dbg_xq = nc.dram_tensor("dbg_xq", [128, 64], BF16, kind="ExternalOutput")
        dbg_qt = nc.dram_tensor("dbg_qt", [64, DIM], BF16, kind="ExternalOutput")
        dbg_qtT = nc.dram_tensor("dbg_qtT", [128, B_LOC * NDC * NH],
                                 BF16, kind="ExternalOutput")
        dbg_ctx = nc.dram_tensor("dbg_ctx", [NH, B_LOC * DIM], BF16,
                                 kind="ExternalOutput")
        dbg_out = nc.dram_tensor("dbg_out", [NH, B_LOC * HD], BF16,
                                 kind="ExternalOutput")

    rg = [list(range(NC))]
    vs_engines = None  # round-robin copy engines, set below

    with tile.TileContext(nc) as tc:
        with (
            tc.tile_pool(name="persist", bufs=1) as pers,
            tc.tile_pool(name="dram", bufs=1, space="DRAM") as dram,
            tc.tile_pool(name="xpool", bufs=6) as xpool,
            tc.tile_pool(name="xTpool", bufs=3) as xTpool,
            tc.tile_pool(name="attn", bufs=8) as apool,
            tc.tile_pool(name="small", bufs=2) as smallp,
            tc.tile_pool(name="ctxsb", bufs=1) as ctxsbp,
            tc.tile_pool(name="pC", bufs=1) as pC,
            tc.tile_pool(name="pCw", bufs=1) as pCw,
            tc.tile_pool(name="tps", bufs=3, space="PSUM") as tps,
            tc.tile_pool(name="scps", bufs=1, space="PSUM") as scps,
                                    tc.tile_pool(name="ctxps", bufs=1, space="PSUM") as ctxps,
            tc.tile_pool(name="sumps", bufs=1, space="PSUM") as sumps,
            tc.tile_pool(name="miscps", bufs=1, space="PSUM") as miscps,
        ):
            fps = miscps
            pCps = miscps
            yps = miscps
            ident = pers.tile([128, 128], BF16)
            make_identity(nc, ident)
            ones_bf = pers.tile([128, 1], BF16)
            nc.vector.memset(ones_bf, 1.0)

            a2a1_in = dram.tile([BSZ, 512], BF16)
            a2a1_out = dram.tile([BSZ, 512], BF16)
            a2a2_in = [dram.tile([NC * HL, DIM], BF16, name=f"a2a2i{b}")
                       for b in range(B_LOC)]
            a2a2_out = [dram.tile([NC * HL, DIM], BF16, name=f"a2a2o{b}")
                        for b in range(B_LOC)]

            # ---------------- Phase A: xq (head-sharded) -> tiny AllToAll
            qtT_all = pers.tile([128, B_LOC * NDC * NH], BF16, name="qtTall")
            qtT = [qtT_all[:, b * NDC * NH:(b + 1) * NDC * NH]
                   for b in range(B_LOC)]
            stage1 = []
            with (
                tc.tile_pool(name="pA", bufs=1) as pA,
                tc.tile_pool(name="pAw", bufs=2) as pAw,
            ):
                xlT_sb = pA.tile([128, NDC * BSZ], BF16)
                nc.sync.dma_start(out=xlT_sb, in_=xlT[:, :])
                wq_pieces = []
                for q in range(4):
                    wq_q = pAw.tile([128, 8 * HL * HD], BF16, tag="wqq",
                                    name=f"wqq{q}")
                    nc.sync.dma_start(
                        out=wq_q, in_=wqT[:, q * 4096:(q + 1) * 4096])
                    wq_pieces.append(wq_q)
                # xq[b, o] for the local 512-wide o-slice; one PSUM chain
                xq_psf = scps.tile([128, 512], F32, tag="sc")
                xq_ps = xq_psf[0:BSZ]
                for q in range(4):
                    for k in range(8):
                        c = q * 8 + k
                        nc.tensor.matmul(
                            xq_ps,
                            xlT_sb[:, c * BSZ:(c + 1) * BSZ],
                            wq_pieces[q][:, k * 512:(k + 1) * 512],
                            start=(c == 0), stop=(c == NDC - 1))
                xq_sb = pA.tile([BSZ, 512], BF16)
                nc.scalar.copy(out=xq_sb, in_=xq_ps)
                d = nc.scalar.dma_start(out=a2a1_in[:, :], in_=xq_sb)
                stage1.append(d)

            cc1 = None
            if not nocc:
                cc1 = nc.gpsimd.collective_compute(
                    "AllToAll", mybir.AluOpType.bypass,
                    ins=[a2a1_in.opt()], outs=[a2a1_out.opt()],
                    replica_groups=rg)
                for d in stage1:
                    add_dep_helper(cc1.ins, d.ins, reason="a2a1 input ready")

            # qtT[b][p=D, c*32+h] = sum_d xq[2r+b, h*128+d] * wk_s[h*128+d, c*128+p]
            with (
                tc.tile_pool(name="qn", bufs=1) as qn,
                tc.tile_pool(name="wkp", bufs=8) as wkp,
            ):
                # wk pieces per kv group, streamed (SP queue, after wqT)
                wk_dmas = []
                wk_pieces = []
                for kv in range(N_KV):
                    wkq = wkp.tile([HD, DIM], BF16, tag="wkp",
                                   name=f"wk{kv}")
                    dk = nc.scalar.dma_start(
                        out=wkq, in_=wk[kv * HD:(kv + 1) * HD, :])

                    wk_dmas.append(dk)
                    wk_pieces.append(wkq)
                xq_loc = qn.tile([B_LOC, DIM], BF16)
                xql_dmas = []
                av = a2a1_out.rearrange("(sq bl) o -> bl sq o", sq=NC)
                for bl in range(B_LOC):
                    d = nc.scalar.dma_start(
                        out=xq_loc[bl:bl + 1].rearrange(
                            "p (sq o) -> p sq o", sq=NC),
                        in_=av[bl])
                    xql_dmas.append(d)
                    if cc1 is not None:
                        add_dep_helper(d.ins, cc1.ins, reason="a2a1 done")
                # xqT2[p=d, 2*hg+bl] via PE transposes
                xqT2_psf = miscps.tile([128, 512], BF16, tag="ctp")
                for c in range(NDC):
                    nc.tensor.transpose(
                        xqT2_psf[:, c * 2:(c + 1) * 2],
                        xq_loc[:, c * 128:(c + 1) * 128],
                        ident[0:B_LOC, 0:B_LOC])
                xqT2_sb = qn.tile([128, NDC * B_LOC], BF16)
                nc.scalar.copy(out=xqT2_sb, in_=xqT2_psf[:, 0:NDC * B_LOC])
                # per kv: qtT chunks [128 D, (c, h, bl)]
                for kv in range(N_KV):
                    if kv % 2 == 0:
                        qt_ps = scps.tile([128, 512], F32, tag="sc")
                    else:
                        qt_ps = miscps.tile([128, 512], F32, tag="ctp")
                    qp = qt_ps.rearrange("p (c h bl) -> p c h bl", c=NDC, h=HL)
                    for c in range(NDC):
                        nc.tensor.matmul(
                            qt_ps[:, c * 8:(c + 1) * 8],
                            wk_pieces[kv][:, c * 128:(c + 1) * 128],
                            xqT2_sb[:, 8 * kv:8 * (kv + 1)],
                            start=True, stop=True)
                    qall = qtT_all.rearrange("p (bl c hh) -> p bl c hh",
                                             bl=B_LOC, c=NDC)
                    nc.scalar.copy(
                        out=qall[:, :, :, 4 * kv:4 * (kv + 1)],
                        in_=qt_ps[:, 0:256].rearrange(
                            "p (c h bl) -> p bl c h", c=NDC, h=HL))
                if debug:
                    for b in range(B_LOC):
                        nc.sync.dma_start(
                            out=dbg_qtT[:, b * NDC * NH:(b + 1) * NDC * NH],
                            in_=qtT[b])

            # ---------------- Phase B: streaming attention per local batch
            cc2 = [None, None]
            if True:
                xdmas = []
                import os
                _CP = os.environ.get("XTCOPY", "3d1a")
                def xt_copy(g, out, in_):
                    # GPSIMD cannot read PSUM; split PSUM->SBUF copies
                    # between DVE and ACT.
                    if _CP == "alldve":
                        nc.vector.tensor_copy(out=out, in_=in_)
                    elif _CP == "2d2a":
                        if g in (1, 3):
                            nc.scalar.copy(out=out, in_=in_)
                        else:
                            nc.vector.tensor_copy(out=out, in_=in_)
                    elif _CP == "split":
                        if g in (1, 3):
                            nc.scalar.copy(out=out[:, 0:512], in_=in_[:, 0:512])
                            nc.vector.tensor_copy(out=out[:, 512:1024],
                                                  in_=in_[:, 512:1024])
                        else:
                            nc.vector.tensor_copy(out=out, in_=in_)
                    else:
                        if g == 1:
                            nc.scalar.copy(out=out, in_=in_)
                        else:
                            nc.vector.tensor_copy(out=out, in_=in_)
                fin_state = {}
                fin2_state = {}

                def finalize_part1(bb, ctx_ps_b, sumT_ps_b):
                    sumT_sb = smallp.tile([NH, 1], F32, tag="ssum")
                    nc.vector.tensor_copy(out=sumT_sb, in_=sumT_ps_b)
                    recip = smallp.tile([NH, 1], F32, tag="recip")
                    nc.vector.reciprocal(out=recip, in_=sumT_sb)
                    ctxT_sb = ctxsbp.tile([128, NDC * NH], BF16, tag="ctxT")
                    for half in range(2):
                        nc.scalar.copy(
                            out=ctxT_sb[:, half * 512:(half + 1) * 512],
                            in_=ctx_ps_b[:, half * 512:(half + 1) * 512])
                    fin_state[bb] = (ctxT_sb, recip)

                def finalize_part2_groups(bb, groups, state):
                    last = (bb == B_LOC - 1)
                    ctxT_sb, recip = fin_state[bb]
                    if "ctx_sb" not in state:
                        state["ctx_sb"] = ctxsbp.tile([NH, DIM], BF16,
                                                      tag="ctxn",
                                                      name=f"ctxn{bb}")
                    ctx_sb = state["ctx_sb"]
                    for g in groups:
                        tp2f = tps.tile([128, 1024], BF16, tag="xtp")
                        tp2 = tp2f[0:NH]
                        for k in range(8):
                            c = g * 8 + k
                            nc.tensor.transpose(
                                tp2[:, k * 128:(k + 1) * 128],
                                ctxT_sb[:, c * NH:(c + 1) * NH],
                                ident)
                        if last and g % 2 == 1:
                            nc.scalar.mul(
                                out=ctx_sb[:, g * 1024:(g + 1) * 1024],
                                in_=tp2, mul=recip)
                        else:
                            nc.vector.tensor_scalar_mul(
                                ctx_sb[:, g * 1024:(g + 1) * 1024], tp2, recip)

                def finalize_part2_finish(bb, state):
                    last = (bb == B_LOC - 1)
                    fin_state.pop(bb)
                    ctx_sb = state["ctx_sb"]
                    if debug:
                        nc.sync.dma_start(
                            out=dbg_ctx[:, bb * DIM:(bb + 1) * DIM],
                            in_=ctx_sb)
                    if last:
                        d = nc.scalar.dma_start(out=a2a2_in[bb][:, :],
                                                in_=ctx_sb)
                    else:
                        d = nc.gpsimd.dma_start(out=a2a2_in[bb][:, :],
                                                in_=ctx_sb)
                    if not nocc:
                        cc2[bb] = nc.gpsimd.collective_compute(
                            "AllToAll", mybir.AluOpType.bypass,
                            ins=[a2a2_in[bb].opt()], outs=[a2a2_out[bb].opt()],
                            replica_groups=rg)
                        add_dep_helper(cc2[bb].ins, d.ins,
                                       reason="a2a2 input ready")

                for b in range(B_LOC):
                    ctx_ps = ctxps.tile([128, NDC * NH], F32, tag="ctx")
                    sumT_ps = sumps.tile([NH, 1], F32, tag="sumT")

                    def emit_attn(tt, x_sb, xT_sb):
                        sc_full = scps.tile([128, 512], F32, tag="sc")
                        sc_ps = sc_full[:, 0:NH]
                        for c in range(NDC):
                            nc.tensor.matmul(
                                sc_ps,
                                xT_sb[:, c * 128:(c + 1) * 128],
                                qtT[b][:, c * NH:(c + 1) * NH],
                                start=(c == 0), stop=(c == NDC - 1))
                        at_sb = apool.tile([128, NH], BF16, tag="at")
                        nc.scalar.activation(
                            out=at_sb, in_=sc_ps,
                            func=mybir.ActivationFunctionType.Exp)
                        nc.tensor.matmul(sumT_ps, at_sb, ones_bf,
                                         start=(tt == 0), stop=(tt == NT - 1))
                        bank_start = [None, None]
                        for c in range(NDC):
                            mm = nc.tensor.matmul(
                                ctx_ps[:, c * NH:(c + 1) * NH],
                                x_sb[:, c * 128:(c + 1) * 128],
                                at_sb,
                                start=(tt == 0 and c % 16 == 0),
                                stop=(tt == NT - 1),
                                skip_group_check=True)
                            if tt == 0:
                                if c % 16 == 0:
                                    bank_start[c // 16] = mm
                                else:
                                    add_dep_helper(
                                        mm.ins, bank_start[c // 16].ins,
                                        reason="bank wipe first")

                    pending = None
                    for tt in range(NT):
                        x_sb = xpool.tile([128, DIM], BF16, tag="x",
                                          name=f"x{b}_{tt}")
                        xd = nc.sync.dma_start(
                            out=x_sb, in_=xp[b, tt * 128:(tt + 1) * 128, :])
                        if len(xdmas) == 2 and stage1:
                            add_dep_helper(xd.ins, stage1[-1].ins,
                                           reason="stage before prefetch")
                        import os as _os2
                        _WKPIN = int(_os2.environ.get("WKPIN", "0"))
                        if _WKPIN and 3 <= len(xdmas) <= 10:
                            add_dep_helper(xd.ins,
                                           wk_dmas[len(xdmas) - 3].ins,
                                           reason="wk pacing")
                        if len(xdmas) in (8, 9) and xql_dmas:
                            add_dep_helper(xd.ins, xql_dmas[-1].ins,
                                           reason="xq_loc priority")
                        xdmas.append(xd)
                        xT_sb = xTpool.tile([128, DIM], BF16, tag="xT")
                        for g in range(0 if notrans else 4):
                            tp = tps.tile([128, 1024], BF16, tag="xtp")
                            for k in range(8):
                                c = g * 8 + k
                                nc.tensor.transpose(
                                    tp[:, k * 128:(k + 1) * 128],
                                    x_sb[:, c * 128:(c + 1) * 128], ident)
                            xt_copy(g, xT_sb[:, g * 1024:(g + 1) * 1024], tp)
                        if noattn:
                            continue
                        if pending is not None:
                            emit_attn(*pending)
                        pending = (tt, x_sb, xT_sb)
                    if pending is not None:
                        emit_attn(*pending)
                    # finalize batch b
                    sumT_sb = smallp.tile([NH, 1], F32, tag="ssum")
                    nc.vector.tensor_copy(out=sumT_sb, in_=sumT_ps)
                    recip = smallp.tile([NH, 1], F32, tag="recip")
                    nc.vector.reciprocal(out=recip, in_=sumT_sb)
                    ctxT_sb = ctxsbp.tile([128, NDC * NH], BF16, tag="ctxT")
                    for half in range(2):
                        nc.scalar.copy(
                            out=ctxT_sb[:, half * 512:(half + 1) * 512],
                            in_=ctx_ps[:, half * 512:(half + 1) * 512])
                    ctx_sb = ctxsbp.tile([NH, DIM], BF16, tag="ctxn")
                    for g in range(4):
                        tp2f = tps.tile([128, 1024], BF16, tag="xtp")
                        tp2 = tp2f[0:NH]
                        for k in range(8):
                            c = g * 8 + k
                            nc.tensor.transpose(
                                tp2[:, k * 128:(k + 1) * 128],
                                ctxT_sb[:, c * NH:(c + 1) * NH],
                                ident)
                        nc.vector.tensor_scalar_mul(
                            ctx_sb[:, g * 1024:(g + 1) * 1024], tp2, recip)
                    if debug:
                        nc.sync.dma_start(
                            out=dbg_ctx[:, b * DIM:(b + 1) * DIM], in_=ctx_sb)
                    d = nc.gpsimd.dma_start(out=a2a2_in[b][:, :], in_=ctx_sb)
                    if not nocc:
                        cc2[b] = nc.gpsimd.collective_compute(
                            "AllToAll", mybir.AluOpType.bypass,
                            ins=[a2a2_in[b].opt()], outs=[a2a2_out[b].opt()],
                            replica_groups=rg)
                        add_dep_helper(cc2[b].ins, d.ins,
                                       reason="a2a2 input ready")

                # ---------------- Phase C: output projection per batch slot
                wvT_sb = pCw.tile([128, NDC * HD], BF16)
                dwv = nc.sync.dma_start(out=wvT_sb, in_=wvT[:, :])
                woT_sb = pCw.tile([128, HL * DIM], BF16)
                dwo = nc.sync.dma_start(out=woT_sb, in_=woT[:, :])
                add_dep_helper(dwv.ins, xdmas[NT + 4].ins, reason="late wv")
                add_dep_helper(dwo.ins, xdmas[NT + 8].ins, reason="late wo")
                yT_sb = pCw.tile([128, NDC * B_LOC * NC], BF16)
                for b in range(B_LOC):
                    ctxgf = xpool.tile([128, DIM], BF16, tag="x",
                                       name=f"ctxg{b}")
                    ctxg = ctxgf[0:NH]
                    d = nc.scalar.dma_start(out=ctxg, in_=a2a2_out[b][:, :])
                    if cc2[b] is not None:
                        add_dep_helper(d.ins, cc2[b].ins, reason="a2a2 done")
                    ctxgT = pC.tile([128, NDC * NH], BF16, tag="ctxgT")
                    tpg = tps.tile([128, 1024], BF16, tag="xtp")
                    for c in range(NDC):
                        nc.tensor.transpose(
                            tpg[:, c * 32:(c + 1) * 32],
                            ctxg[:, c * 128:(c + 1) * 128],
                            ident[0:NH, 0:NH])
                    nc.vector.tensor_copy(out=ctxgT, in_=tpg)
                    # outT[d, (s,h)] = sum_D wvT[D, d]^T ctxgT[D, (s,h)]
                    op_ps = pCps.tile([HD, NH], F32, tag="ctp")
                    for c in range(NDC):
                        nc.tensor.matmul(op_ps,
                                         wvT_sb[:, c * 128:(c + 1) * 128],
                                         ctxgT[:, c * NH:(c + 1) * NH],
                                         start=(c == 0), stop=(c == NDC - 1))
                    outT = pC.tile([128, NH], BF16, tag="outT")
                    nc.vector.tensor_copy(out=outT[0:HD], in_=op_ps)
                    # yT[j, s] = sum_h sum_d woT[d, (h, jc, j)] * outT[d, (s, h)]
                    ov = outT.rearrange("p (s h) -> p h s", h=HL)
                    y_ps = yps.tile([128, NDC * NC], F32, tag="ctp")
                    for jc in range(NDC):
                        for h in range(HL):
                            nc.tensor.matmul(
                                y_ps[:, jc * NC:(jc + 1) * NC],
                                woT_sb[:, h * DIM + jc * 128:
                                       h * DIM + (jc + 1) * 128],
                                ov[:, h, :],
                                start=(h == 0), stop=(h == HL - 1))
                    yv = yT_sb.rearrange("p (b jcs) -> b p jcs", b=B_LOC)
                    nc.vector.tensor_copy(out=yv[b], in_=y_ps)
                    nc.sync.dma_start(
                        out=yT.rearrange("p (b jcs) -> b p jcs", b=B_LOC)[b],
                        in_=yv[b])

    nc.finalize()
    return nc


_PROGRAM_CACHE = {}


def _prep_inputs(x_pre, wq, wk, wv, wo):
    """Shard + cast + pre-transpose on host. Returns in_maps for 8 cores."""
    xlT_full = np.ascontiguousarray(
        x_pre[:, -1, :].T.astype(NPBF))                    # [4096, 16]
    xlT_full = xlT_full.reshape(NDC, 128, BSZ).transpose(1, 0, 2)  # [128,c,b]
    xlT_flat = np.ascontiguousarray(xlT_full.reshape(128, NDC * BSZ))

    wk_s = (wk * SCALE).astype(NPBF)
    in_maps = []
    for r in range(NC):
        # wqT[p, c, h, o] = wq[512r + h*128 + o, c*128 + p]
        wq_sl = wq[512 * r:512 * (r + 1), :].astype(NPBF)   # [512, 4096] (h,o)xD
        wqT_r = wq_sl.reshape(HL, 128, NDC, 128).transpose(3, 2, 0, 1)
        wqT_r = np.ascontiguousarray(wqT_r.reshape(128, NDC * HL * HD))
        # wvT[p, c, d] = wv[128r + d, c*128 + p]
        wv_sl = wv[128 * r:128 * (r + 1), :].astype(NPBF)   # [128 d, 4096 D]
        wvT_r = wv_sl.reshape(128, NDC, 128).transpose(2, 1, 0)
        wvT_r = np.ascontiguousarray(wvT_r.reshape(128, NDC * HD))
        # woT[p, h, jc, j] = wo[jc*128 + j, 512r + h*128 + p]
        wo_sl = wo[:, 512 * r:512 * (r + 1)].astype(NPBF)   # [4096 j, 512 o]
        woT_r = wo_sl.reshape(NDC, 128, HL, 128).transpose(3, 2, 0, 1)
        woT_r = np.ascontiguousarray(woT_r.reshape(128, HL * DIM))
        in_maps.append({
            "xp": np.ascontiguousarray(x_pre[2 * r:2 * r + 2].astype(NPBF)),
            "xlT": xlT_flat,
            "wqT": wqT_r,
            "wk": np.ascontiguousarray(wk_s),
            "wvT": wvT_r,
            "woT": woT_r,
        })
    return in_maps


def kernel(x_pre, wq, wk, wv, wo, _trace=False, _tmpdir=None, _debug=False):
    x_pre = np.asarray(x_pre, dtype=np.float32)
    wq = np.asarray(wq, dtype=np.float32)
    wk = np.asarray(wk, dtype=np.float32)
    wv = np.asarray(wv, dtype=np.float32)
    wo = np.asarray(wo, dtype=np.float32)

    key = "nc_dbg" if _debug else "nc"
    if key not in _PROGRAM_CACHE:
        _PROGRAM_CACHE[key] = build_program(debug=_debug)
        _PROGRAM_CACHE["nc"] = _PROGRAM_CACHE[key]
    nc = _PROGRAM_CACHE[key]

    in_maps = _prep_inputs(x_pre, wq, wk, wv, wo)

    kwargs = {}
    if _trace:
        kwargs = dict(trace=True, trace_cores=[0])
    if _tmpdir is not None:
        kwargs["tmpdir"] = _tmpdir
    res = run_bass_kernel_spmd(nc, in_maps, core_ids=list(range(NC)), **kwargs)

    y = np.zeros((BSZ, DIM), np.float64)
    for r in range(NC):
        yT_r = np.asarray(res.results[r]["yT"], np.float32)
        yT_r = yT_r.reshape(128, B_LOC, NDC, NC)
        # y[2s+b, jc*128+p] += yT_r[p, b, jc, s]
        y += yT_r.transpose(3, 1, 2, 0).reshape(BSZ, DIM)
    if _debug:
        _PROGRAM_CACHE["dbg"] = res
    if _trace:
        print("HW exec time:", res.exec_time_ns, "ns")
    return y.astype(np.float32).reshape(BSZ, 1, DIM)


# revision 65
# speedup vs baseline: 4.9060x; 1.0189x over previous
"""Bass/Trainium2 kernel for GQA decode attention (fused K-projection form).

Reference computation:
  x = x_pre[:, -1, :]                               # [16, 4096]
  xq = (x @ wq.T) -> [b, 32, 128]
  qt[b,h,:] = xq[b,h,:] @ wk[kv(h)*128:+128, :]     # [b, 32, 4096]
  scores = qt . x_pre / sqrt(128)                   # [b, 32, 2048]
  attn = softmax_t(scores)
  ctx[b,h,:] = sum_t attn[b,h,t] * x_pre[b,t,:]     # [b, 32, 4096]  (lazy-V)
  out[b,h,d] = sum_D ctx[b,h,D] * wv[kv(h)*128+d,D] # [b, 32, 128]
  y = out.flat @ wo.T                               # [16, 4096]

Sharding (8 cores): batch-parallel attention (2 batches/core) +
head-parallel projections (4 heads = 1 kv group/core), exchanged with
AllToAll collectives.  All device data is bf16 (f32 PSUM accumulation);
weights are pre-transposed on the host into the layouts the PE consumes,
and the big matmuls are arranged stationary-heavy (large lhsT, narrow
moving operand) so PE streaming cost is minimized.
"""

import math

import numpy as np
import ml_dtypes

import concourse.bass as bass
import concourse.mybir as mybir
import concourse.tile as tile
from concourse import bacc
from concourse.bass_utils import run_bass_kernel_spmd
from concourse.masks import make_identity
from concourse.tile import add_dep_helper

F32 = mybir.dt.float32
BF16 = mybir.dt.bfloat16
NPBF = ml_dtypes.bfloat16

NC = 8
BSZ = 16
SEQ = 2048
DIM = 4096
NH = 32
HD = 128
B_LOC = 2        # batches per core
HL = 4           # local heads per core (= one kv group)
N_KV = 8
NT = SEQ // 128  # 16 t-tiles per batch
NDC = DIM // 128 # 32 D-chunks
SCALE = 1.0 / math.sqrt(HD)


def build_program(debug=False, nocc=False, noattn=False, notrans=False, nocopy=False):
    nc = bacc.Bacc("TRN2", target_bir_lowering=False, debug=False)

    xp = nc.dram_tensor("xp", [B_LOC, SEQ, DIM], BF16, kind="ExternalInput")
    # xlT[p, c, b] = x_pre[b, -1, c*128+p]
    xlT = nc.dram_tensor("xlT", [128, NDC * BSZ], BF16, kind="ExternalInput")
    # wqT[p, c*512 + h*128 + o] = wq[512r + h*128 + o, c*128 + p]
    wqT = nc.dram_tensor("wqT", [128, NDC * HL * HD], BF16,
                         kind="ExternalInput")
    # wk_s = wk * SCALE  (full, natural [kv*128+d, D])
    wk = nc.dram_tensor("wk", [N_KV * HD, DIM], BF16, kind="ExternalInput")
    # wvT[p, c*128 + d] = wv[128r + d, c*128 + p]
    wvT = nc.dram_tensor("wvT", [128, NDC * HD], BF16, kind="ExternalInput")
    # woT[p, h*4096 + jc*128 + j] = wo[jc*128 + j, 512r + h*128 + p]
    woT = nc.dram_tensor("woT", [128, HL * DIM], BF16, kind="ExternalInput")
    # yT[p, b*256 + jc*8 + s] = y_partial[2s+b, jc*128+p]
    yT = nc.dram_tensor("yT", [128, B_LOC * NDC * NC], BF16,
                        kind="ExternalOutput")
    if debug:
        dbg_xq = nc.dram_tensor("dbg_xq", [128, 64], BF16, kind="ExternalOutput")
        dbg_qt = nc.dram_tensor("dbg_qt", [64, DIM], BF16, kind="ExternalOutput")
        dbg_qtT = nc.dram_tensor("dbg_qtT", [128, B_LOC * NDC * NH],
                                 BF16, kind="ExternalOutput")
        dbg_ctx = nc.dram_tensor("dbg_ctx", [NH, B_LOC * DIM], BF16,
                                 kind="ExternalOutput")
        dbg_out = nc.dram_tensor("dbg_out", [NH, B_LOC * HD], BF16,
                                 kind="ExternalOutput")

    rg = [list(range(NC))]
    vs_engines = None  # round-robin copy engines, set below

    with tile.TileContext(nc) as tc:
        with (
            tc.tile_pool(name="persist", bufs=1) as pers,
            tc.tile_pool(name="dram", bufs=1, space="DRAM") as dram,
            tc.tile_pool(name="xpool", bufs=6) as xpool,
            tc.tile_pool(name="xTpool", bufs=3) as xTpool,
            tc.tile_pool(name="attn", bufs=8) as apool,
            tc.tile_pool(name="small", bufs=2) as smallp,
            tc.tile_pool(name="ctxsb", bufs=1) as ctxsbp,
            tc.tile_pool(name="pC", bufs=1) as pC,
            tc.tile_pool(name="pCw", bufs=1) as pCw,
            tc.tile_pool(name="tps", bufs=3, space="PSUM") as tps,
            tc.tile_pool(name="scps", bufs=1, space="PSUM") as scps,
                                    tc.tile_pool(name="ctxps", bufs=1, space="PSUM") as ctxps,
            tc.tile_pool(name="sumps", bufs=1, space="PSUM") as sumps,
            tc.tile_pool(name="miscps", bufs=1, space="PSUM") as miscps,
        ):
            fps = miscps
            pCps = miscps
            yps = miscps
            ident = pers.tile([128, 128], BF16)
            make_identity(nc, ident)
            ones_bf = pers.tile([128, 1], BF16)
            nc.vector.memset(ones_bf, 1.0)

            a2a1_in = dram.tile([BSZ, 512], BF16)
            a2a1_out = dram.tile([BSZ, 512], BF16)
            a2a2_in = [dram.tile([NC * HL, DIM], BF16, name=f"a2a2i{b}")
                       for b in range(B_LOC)]
            a2a2_out = [dram.tile([NC * HL, DIM], BF16, name=f"a2a2o{b}")
                        for b in range(B_LOC)]

            # ---------------- Phase A: xq (head-sharded) -> tiny AllToAll
            qtT_all = pers.tile([128, B_LOC * NDC * NH], BF16, name="qtTall")
            qtT = [qtT_all[:, b * NDC * NH:(b + 1) * NDC * NH]
                   for b in range(B_LOC)]
            stage1 = []
            with (
                tc.tile_pool(name="pA", bufs=1) as pA,
                tc.tile_pool(name="pAw", bufs=2) as pAw,
            ):
                xlT_sb = pA.tile([128, NDC * BSZ], BF16)
                nc.sync.dma_start(out=xlT_sb, in_=xlT[:, :])
                wq_pieces = []
                for q in range(4):
                    wq_q = pAw.tile([128, 8 * HL * HD], BF16, tag="wqq",
                                    name=f"wqq{q}")
                    nc.sync.dma_start(
                        out=wq_q, in_=wqT[:, q * 4096:(q + 1) * 4096])
                    wq_pieces.append(wq_q)
                # xq[b, o] for the local 512-wide o-slice; one PSUM chain
                xq_psf = scps.tile([128, 512], F32, tag="sc")
                xq_ps = xq_psf[0:BSZ]
                for q in range(4):
                    for k in range(8):
                        c = q * 8 + k
                        nc.tensor.matmul(
                            xq_ps,
                            xlT_sb[:, c * BSZ:(c + 1) * BSZ],
                            wq_pieces[q][:, k * 512:(k + 1) * 512],
                            start=(c == 0), stop=(c == NDC - 1))
                xq_sb = pA.tile([BSZ, 512], BF16)
                nc.scalar.copy(out=xq_sb, in_=xq_ps)
                d = nc.scalar.dma_start(out=a2a1_in[:, :], in_=xq_sb)
                stage1.append(d)

            cc1 = None
            if not nocc:
                cc1 = nc.gpsimd.collective_compute(
                    "AllToAll", mybir.AluOpType.bypass,
                    ins=[a2a1_in.opt()], outs=[a2a1_out.opt()],
                    replica_groups=rg)
                for d in stage1:
                    add_dep_helper(cc1.ins, d.ins, reason="a2a1 input ready")

            # qtT[b][p=D, c*32+h] = sum_d xq[2r+b, h*128+d] * wk_s[h*128+d, c*128+p]
            with (
                tc.tile_pool(name="qn", bufs=1) as qn,
                tc.tile_pool(name="wkp", bufs=8) as wkp,
            ):
                # wk pieces per kv group, streamed (SP queue, after wqT)
                wk_dmas = []
                wk_pieces = []
                for kv in range(N_KV):
                    wkq = wkp.tile([HD, DIM], BF16, tag="wkp",
                                   name=f"wk{kv}")
                    dk = nc.sync.dma_start(
                        out=wkq, in_=wk[kv * HD:(kv + 1) * HD, :])

                    wk_dmas.append(dk)
                    wk_pieces.append(wkq)
                xq_loc = qn.tile([B_LOC, DIM], BF16)
                xql_dmas = []
                av = a2a1_out.rearrange("(sq bl) o -> bl sq o", sq=NC)
                for bl in range(B_LOC):
                    d = nc.scalar.dma_start(
                        out=xq_loc[bl:bl + 1].rearrange(
                            "p (sq o) -> p sq o", sq=NC),
                        in_=av[bl])
                    xql_dmas.append(d)
                    if cc1 is not None:
                        add_dep_helper(d.ins, cc1.ins, reason="a2a1 done")
                # xqT2[p=d, 2*hg+bl] via PE transposes
                xqT2_psf = miscps.tile([128, 512], BF16, tag="ctp")
                for c in range(NDC):
                    nc.tensor.transpose(
                        xqT2_psf[:, c * 2:(c + 1) * 2],
                        xq_loc[:, c * 128:(c + 1) * 128],
                        ident[0:B_LOC, 0:B_LOC])
                xqT2_sb = qn.tile([128, NDC * B_LOC], BF16)
                nc.scalar.copy(out=xqT2_sb, in_=xqT2_psf[:, 0:NDC * B_LOC])
                # per kv: qtT chunks [128 D, (c, h, bl)]
                for kv in range(N_KV):
                    if kv % 2 == 0:
                        qt_ps = scps.tile([128, 512], F32, tag="sc")
                    else:
                        qt_ps = miscps.tile([128, 512], F32, tag="ctp")
                    qp = qt_ps.rearrange("p (c h bl) -> p c h bl", c=NDC, h=HL)
                    for c in range(NDC):
                        nc.tensor.matmul(
                            qt_ps[:, c * 8:(c + 1) * 8],
                            wk_pieces[kv][:, c * 128:(c + 1) * 128],
                            xqT2_sb[:, 8 * kv:8 * (kv + 1)],
                            start=True, stop=True)
                    qall = qtT_all.rearrange("p (bl c hh) -> p bl c hh",
                                             bl=B_LOC, c=NDC)
                    nc.scalar.copy(
                        out=qall[:, :, :, 4 * kv:4 * (kv + 1)],
                        in_=qt_ps[:, 0:256].rearrange(
                            "p (c h bl) -> p bl c h", c=NDC, h=HL))
                if debug:
                    for b in range(B_LOC):
                        nc.sync.dma_start(
                            out=dbg_qtT[:, b * NDC * NH:(b + 1) * NDC * NH],
                            in_=qtT[b])

            # ---------------- Phase B: streaming attention per local batch
            cc2 = [None, None]
            if True:
                xdmas = []
                import os
                _CP = os.environ.get("XTCOPY", "3d1a")
                def xt_copy(g, out, in_):
                    # GPSIMD cannot read PSUM; split PSUM->SBUF copies
                    # between DVE and ACT.
                    if _CP == "alldve":
                        nc.vector.tensor_copy(out=out, in_=in_)
                    elif _CP == "2d2a":
                        if g in (1, 3):
                            nc.scalar.copy(out=out, in_=in_)
                        else:
                            nc.vector.tensor_copy(out=out, in_=in_)
                    elif _CP == "split":
                        if g in (1, 3):
                            nc.scalar.copy(out=out[:, 0:512], in_=in_[:, 0:512])
                            nc.vector.tensor_copy(out=out[:, 512:1024],
                                                  in_=in_[:, 512:1024])
                        else:
                            nc.vector.tensor_copy(out=out, in_=in_)
                    else:
                        if g == 1:
                            nc.scalar.copy(out=out, in_=in_)
                        else:
                            nc.vector.tensor_copy(out=out, in_=in_)
                fin_state = {}
                fin2_state = {}

                def finalize_part1(bb, ctx_ps_b, sumT_ps_b):
                    sumT_sb = smallp.tile([NH, 1], F32, tag="ssum")
                    nc.vector.tensor_copy(out=sumT_sb, in_=sumT_ps_b)
                    recip = smallp.tile([NH, 1], F32, tag="recip")
                    nc.vector.reciprocal(out=recip, in_=sumT_sb)
                    ctxT_sb = ctxsbp.tile([128, NDC * NH], BF16, tag="ctxT")
                    for half in range(2):
                        nc.scalar.copy(
                            out=ctxT_sb[:, half * 512:(half + 1) * 512],
                            in_=ctx_ps_b[:, half * 512:(half + 1) * 512])
                    fin_state[bb] = (ctxT_sb, recip)

                def finalize_part2_groups(bb, groups, state):
                    last = (bb == B_LOC - 1)
                    ctxT_sb, recip = fin_state[bb]
                    if "ctx_sb" not in state:
                        state["ctx_sb"] = ctxsbp.tile([NH, DIM], BF16,
                                                      tag="ctxn",
                                                      name=f"ctxn{bb}")
                    ctx_sb = state["ctx_sb"]
                    for g in groups:
                        tp2f = tps.tile([128, 1024], BF16, tag="xtp")
                        tp2 = tp2f[0:NH]
                        for k in range(8):
                            c = g * 8 + k
                            nc.tensor.transpose(
                                tp2[:, k * 128:(k + 1) * 128],
                                ctxT_sb[:, c * NH:(c + 1) * NH],
                                ident)
                        if last and g % 2 == 1:
                            nc.scalar.mul(
                                out=ctx_sb[:, g * 1024:(g + 1) * 1024],
                                in_=tp2, mul=recip)
                        else:
                            nc.vector.tensor_scalar_mul(
                                ctx_sb[:, g * 1024:(g + 1) * 1024], tp2, recip)

                def finalize_part2_finish(bb, state):
                    last = (bb == B_LOC - 1)
                    fin_state.pop(bb)
                    ctx_sb = state["ctx_sb"]
                    if debug:
                        nc.sync.dma_start(
                            out=dbg_ctx[:, bb * DIM:(bb + 1) * DIM],
                            in_=ctx_sb)
                    if last:
                        d = nc.scalar.dma_start(out=a2a2_in[bb][:, :],
                                                in_=ctx_sb)
                    else:
                        d = nc.gpsimd.dma_start(out=a2a2_in[bb][:, :],
                                                in_=ctx_sb)
                    if not nocc:
                        cc2[bb] = nc.gpsimd.collective_compute(
                            "AllToAll", mybir.AluOpType.bypass,
                            ins=[a2a2_in[bb].opt()], outs=[a2a2_out[bb].opt()],
                            replica_groups=rg)
                        add_dep_helper(cc2[bb].ins, d.ins,
                                       reason="a2a2 input ready")

                for b in range(B_LOC):
                    ctx_ps = ctxps.tile([128, NDC * NH], F32, tag="ctx")
                    sumT_ps = sumps.tile([NH, 1], F32, tag="sumT")

                    def emit_attn(tt, x_sb, xT_sb):
                        sc_full = scps.tile([128, 512], F32, tag="sc")
                        sc_ps = sc_full[:, 0:NH]
                        for c in range(NDC):
                            nc.tensor.matmul(
                                sc_ps,
                                xT_sb[:, c * 128:(c + 1) * 128],
                                qtT[b][:, c * NH:(c + 1) * NH],
                                start=(c == 0), stop=(c == NDC - 1))
                        at_sb = apool.tile([128, NH], BF16, tag="at")
                        nc.scalar.activation(
                            out=at_sb, in_=sc_ps,
                            func=mybir.ActivationFunctionType.Exp)
                        nc.tensor.matmul(sumT_ps, at_sb, ones_bf,
                                         start=(tt == 0), stop=(tt == NT - 1))
                        bank_start = [None, None]
                        for c in range(NDC):
                            mm = nc.tensor.matmul(
                                ctx_ps[:, c * NH:(c + 1) * NH],
                                x_sb[:, c * 128:(c + 1) * 128],
                                at_sb,
                                start=(tt == 0 and c % 16 == 0),
                                stop=(tt == NT - 1),
                                skip_group_check=True)
                            if tt == 0:
                                if c % 16 == 0:
                                    bank_start[c // 16] = mm
                                else:
                                    add_dep_helper(
                                        mm.ins, bank_start[c // 16].ins,
                                        reason="bank wipe first")

                    pending = None
                    for tt in range(NT):
                        x_sb = xpool.tile([128, DIM], BF16, tag="x",
                                          name=f"x{b}_{tt}")
                        xd = nc.sync.dma_start(
                            out=x_sb, in_=xp[b, tt * 128:(tt + 1) * 128, :])
                        import os as _os2
                        _XH = _os2.environ.get("XHOLD", "wk")
                        if len(xdmas) == 0 and _XH == "xql" and xql_dmas:
                            add_dep_helper(xd.ins, xql_dmas[-1].ins,
                                           reason="weights+xql first")
                        elif len(xdmas) == 0 and _XH == "wk" and wk_dmas:
                            add_dep_helper(xd.ins, wk_dmas[-1].ins,
                                           reason="wk first")
                        xdmas.append(xd)
                        xT_sb = xTpool.tile([128, DIM], BF16, tag="xT")
                        for g in range(0 if notrans else 4):
                            tp = tps.tile([128, 1024], BF16, tag="xtp")
                            for k in range(8):
                                c = g * 8 + k
                                nc.tensor.transpose(
                                    tp[:, k * 128:(k + 1) * 128],
                                    x_sb[:, c * 128:(c + 1) * 128], ident)
                            xt_copy(g, xT_sb[:, g * 1024:(g + 1) * 1024], tp)
                        if noattn:
                            continue
                        if pending is not None:
                            emit_attn(*pending)
                        pending = (tt, x_sb, xT_sb)
                    if pending is not None:
                        emit_attn(*pending)
                    # finalize batch b
                    sumT_sb = smallp.tile([NH, 1], F32, tag="ssum")
                    nc.vector.tensor_copy(out=sumT_sb, in_=sumT_ps)
                    recip = smallp.tile([NH, 1], F32, tag="recip")
                    nc.vector.reciprocal(out=recip, in_=sumT_sb)
                    ctxT_sb = ctxsbp.tile([128, NDC * NH], BF16, tag="ctxT")
                    for half in range(2):
                        nc.scalar.copy(
                            out=ctxT_sb[:, half * 512:(half + 1) * 512],
                            in_=ctx_ps[:, half * 512:(half + 1) * 512])
                    ctx_sb = ctxsbp.tile([NH, DIM], BF16, tag="ctxn")
                    for g in range(4):
                        tp2f = tps.tile([128, 1024], BF16, tag="xtp")
                        tp2 = tp2f[0:NH]
                        for k in range(8):
                            c = g * 8 + k
                            nc.tensor.transpose(
                                tp2[:, k * 128:(k + 1) * 128],
                                ctxT_sb[:, c * NH:(c + 1) * NH],
                                ident)
                        nc.vector.tensor_scalar_mul(
                            ctx_sb[:, g * 1024:(g + 1) * 1024], tp2, recip)
                    if debug:
                        nc.sync.dma_start(
                            out=dbg_ctx[:, b * DIM:(b + 1) * DIM], in_=ctx_sb)
                    d = nc.gpsimd.dma_start(out=a2a2_in[b][:, :], in_=ctx_sb)
                    if not nocc:
                        cc2[b] = nc.gpsimd.collective_compute(
                            "AllToAll", mybir.AluOpType.bypass,
                            ins=[a2a2_in[b].opt()], outs=[a2a2_out[b].opt()],
                            replica_groups=rg)
                        add_dep_helper(cc2[b].ins, d.ins,
                                       reason="a2a2 input ready")

                # ---------------- Phase C: output projection per batch slot
                wvT_sb = pCw.tile([128, NDC * HD], BF16)
                dwv = nc.sync.dma_start(out=wvT_sb, in_=wvT[:, :])
                woT_sb = pCw.tile([128, HL * DIM], BF16)
                dwo = nc.sync.dma_start(out=woT_sb, in_=woT[:, :])
                add_dep_helper(dwv.ins, xdmas[NT + 4].ins, reason="late wv")
                add_dep_helper(dwo.ins, xdmas[NT + 8].ins, reason="late wo")
                yT_sb = pCw.tile([128, NDC * B_LOC * NC], BF16)
                for b in range(B_LOC):
                    ctxgf = xpool.tile([128, DIM], BF16, tag="x",
                                       name=f"ctxg{b}")
                    ctxg = ctxgf[0:NH]
                    d = nc.scalar.dma_start(out=ctxg, in_=a2a2_out[b][:, :])
                    if cc2[b] is not None:
                        add_dep_helper(d.ins, cc2[b].ins, reason="a2a2 done")
                    ctxgT = pC.tile([128, NDC * NH], BF16, tag="ctxgT")
                    tpg = tps.tile([128, 1024], BF16, tag="xtp")
                    for c in range(NDC):
                        nc.tensor.transpose(
                            tpg[:, c * 32:(c + 1) * 32],
                            ctxg[:, c * 128:(c + 1) * 128],
                            ident[0:NH, 0:NH])
                    nc.vector.tensor_copy(out=ctxgT, in_=tpg)
                    # outT[d, (s,h)] = sum_D wvT[D, d]^T ctxgT[D, (s,h)]
                    op_ps = pCps.tile([HD, NH], F32, tag="ctp")
                    for c in range(NDC):
                        nc.tensor.matmul(op_ps,
                                         wvT_sb[:, c * 128:(c + 1) * 128],
                                         ctxgT[:, c * NH:(c + 1) * NH],
                                         start=(c == 0), stop=(c == NDC - 1))
                    outT = pC.tile([128, NH], BF16, tag="outT")
                    nc.vector.tensor_copy(out=outT[0:HD], in_=op_ps)
                    # yT[j, s] = sum_h sum_d woT[d, (h, jc, j)] * outT[d, (s, h)]
                    ov = outT.rearrange("p (s h) -> p h s", h=HL)
                    y_ps = yps.tile([128, NDC * NC], F32, tag="ctp")
                    for jc in range(NDC):
                        for h in range(HL):
                            nc.tensor.matmul(
                                y_ps[:, jc * NC:(jc + 1) * NC],
                                woT_sb[:, h * DIM + jc * 128:
                                       h * DIM + (jc + 1) * 128],
                                ov[:, h, :],
                                start=(h == 0), stop=(h == HL - 1))
                    yv = yT_sb.rearrange("p (b jcs) -> b p jcs", b=B_LOC)
                    nc.vector.tensor_copy(out=yv[b], in_=y_ps)
                    nc.sync.dma_start(
                        out=yT.rearrange("p (b jcs) -> b p jcs", b=B_LOC)[b],
                        in_=yv[b])

    nc.finalize()
    return nc


_PROGRAM_CACHE = {}


def _prep_inputs(x_pre, wq, wk, wv, wo):
    """Shard + cast + pre-transpose on host. Returns in_maps for 8 cores."""
    xlT_full = np.ascontiguousarray(
        x_pre[:, -1, :].T.astype(NPBF))                    # [4096, 16]
    xlT_full = xlT_full.reshape(NDC, 128, BSZ).transpose(1, 0, 2)  # [128,c,b]
    xlT_flat = np.ascontiguousarray(xlT_full.reshape(128, NDC * BSZ))

    wk_s = (wk * SCALE).astype(NPBF)
    in_maps = []
    for r in range(NC):
        # wqT[p, c, h, o] = wq[512r + h*128 + o, c*128 + p]
        wq_sl = wq[512 * r:512 * (r + 1), :].astype(NPBF)   # [512, 4096] (h,o)xD
        wqT_r = wq_sl.reshape(HL, 128, NDC, 128).transpose(3, 2, 0, 1)
        wqT_r = np.ascontiguousarray(wqT_r.reshape(128, NDC * HL * HD))
        # wvT[p, c, d] = wv[128r + d, c*128 + p]
        wv_sl = wv[128 * r:128 * (r + 1), :].astype(NPBF)   # [128 d, 4096 D]
        wvT_r = wv_sl.reshape(128, NDC, 128).transpose(2, 1, 0)
        wvT_r = np.ascontiguousarray(wvT_r.reshape(128, NDC * HD))
        # woT[p, h, jc, j] = wo[jc*128 + j, 512r + h*128 + p]
        wo_sl = wo[:, 512 * r:512 * (r + 1)].astype(NPBF)   # [4096 j, 512 o]
        woT_r = wo_sl.reshape(NDC, 128, HL, 128).transpose(3, 2, 0, 1)
        woT_r = np.ascontiguousarray(woT_r.reshape(128, HL * DIM))
        in_maps.append({
            "xp": np.ascontiguousarray(x_pre[2 * r:2 * r + 2].astype(NPBF)),
            "xlT": xlT_flat,
            "wqT": wqT_r,
            "wk": np.ascontiguousarray(wk_s),
            "wvT": wvT_r,
            "woT": woT_r,
        })
    return in_maps


def kernel(x_pre, wq, wk, wv, wo, _trace=False, _tmpdir=None, _debug=False):
    x_pre = np.asarray(x_pre, dtype=np.float32)
    wq = np.asarray(wq, dtype=np.float32)
    wk = np.asarray(wk, dtype=np.float32)
    wv = np.asarray(wv, dtype=np.float32)
    wo = np.asarray(wo, dtype=np.float32)

    key = "nc_dbg" if _debug else "nc"
    if key not in _PROGRAM_CACHE:
        _PROGRAM_CACHE[key] = build_program(debug=_debug)
        _PROGRAM_CACHE["nc"] = _PROGRAM_CACHE[key]
    nc = _PROGRAM_CACHE[key]

    in_maps = _prep_inputs(x_pre, wq, wk, wv, wo)

    kwargs = {}
    if _trace:
        kwargs = dict(trace=True, trace_cores=[0])
    if _tmpdir is not None:
        kwargs["tmpdir"] = _tmpdir
    res = run_bass_kernel_spmd(nc, in_maps, core_ids=list(range(NC)), **kwargs)

    y = np.zeros((BSZ, DIM), np.float64)
    for r in range(NC):
        yT_r = np.asarray(res.results[r]["yT"], np.float32)
        yT_r = yT_r.reshape(128, B_LOC, NDC, NC)
        # y[2s+b, jc*128+p] += yT_r[p, b, jc, s]
        y += yT_r.transpose(3, 1, 2, 0).reshape(BSZ, DIM)
    if _debug:
        _PROGRAM_CACHE["dbg"] = res
    if _trace:
        print("HW exec time:", res.exec_time_ns, "ns")
    return y.astype(np.float32).reshape(BSZ, 1, DIM)
